# revision 3
# baseline (speedup 1.0000x reference)
import os
import sys

for _p in ("/opt/trn_rl_repo", "/root/.axon_site/_ro/trn_rl_repo"):
    if os.path.isdir(_p) and _p not in sys.path:
        sys.path.append(_p)

import numpy as np
import concourse.bass as bass
import concourse.mybir as mybir
import concourse.tile as tile
from concourse import masks
from concourse.bass_utils import run_bass_kernel_spmd

# Problem shapes (hardcoded per contract)
BS, IMG, CLS, DIM, WDIM = 16, 196, 80, 512, 300
N_CORES = 8
BPC = BS // N_CORES  # batches per core
NCH = DIM // 128  # 4 d-chunks of 128
SCH = [(0, 128), (128, 68)]  # s-chunks (offset, size) for IMG=196
WCH = [(0, 128), (128, 128), (256, 44)]  # w-chunks for WDIM=300

F32 = mybir.dt.float32
# dtype used for the fc3 matmul operands (W3T weights and prodT moving data)
MM_DT = mybir.dt.float32


def split_multi_waits(nc):
    """This walrus build accepts a single sync-wait per instruction on the
    CTRL encodings; split extra waits into single-wait NoOps on the same
    engine immediately before the instruction."""
    k = 0
    for f in nc.m.functions:
        for bb in f.blocks:
            il = bb.instructions
            i = 0
            while i < len(il):
                ins = il[i]
                si = ins.sync_info
                if si is not None and len(si.on_wait) > 1:
                    waits = list(si.on_wait)
                    for w in waits[:-1]:
                        nop = mybir.InstNoOp(name=f"waitsplit_{k}", ins=[], outs=[])
                        k += 1
                        nop.engine = ins.engine
                        nop.sync_info = mybir.SyncInfo(on_wait=[w], on_update=[])
                        il.insert(i, nop)
                        i += 1
                    ins.sync_info = mybir.SyncInfo(
                        on_wait=[waits[-1]], on_update=list(si.on_update)
                    )
                i += 1
    return k


def build_kernel(mm_dt=MM_DT):
    nc = bass.Bass("TRN2", target_bir_lowering=False, debug=False, num_devices=N_CORES)
    token_d = nc.dram_tensor("token", [BPC, IMG, DIM], F32, kind="ExternalInput").ap()
    lg_d = nc.dram_tensor("lg", [BPC, CLS, WDIM], F32, kind="ExternalInput").ap()
    w1_d = nc.dram_tensor("W1", [DIM, WDIM], F32, kind="ExternalInput").ap()
    w3_d = nc.dram_tensor("W3", [DIM, DIM], F32, kind="ExternalInput").ap()
    out_d = nc.dram_tensor("out", [BPC, IMG, DIM], F32, kind="ExternalOutput").ap()

    with tile.TileContext(nc) as tc:
        emit_body(tc, nc, token_d, lg_d, w1_d, w3_d, out_d, mm_dt)
    split_multi_waits(nc)
    return nc


def emit_body(tc, nc, token_d, lg_d, w1_d, w3_d, out_d, mm_dt):
    TS = mybir.ActivationFunctionType
    AO = mybir.AluOpType

    with tc.tile_pool(name="const", bufs=1) as const_pool:
        _emit_body_inner(tc, nc, token_d, lg_d, w1_d, w3_d, out_d, mm_dt, const_pool)


def _emit_body_inner(tc, nc, token_d, lg_d, w1_d, w3_d, out_d, mm_dt, const_pool):
    TS = mybir.ActivationFunctionType
    AO = mybir.AluOpType

    ident = const_pool.tile([128, 128], F32)
    masks.make_identity(nc, ident[:])

    # Persistent SBUF tensors
    w3t = [const_pool.tile([128, DIM], mm_dt, tag=f"w3t{k}", name=f"w3t{k}") for k in range(NCH)]
    w1t = [const_pool.tile([128, DIM], F32, tag=f"w1t{w}", name=f"w1t{w}") for w in range(len(WCH))]
    tokT = [
        [const_pool.tile([128, IMG], F32, tag=f"tokT{b}_{k}", name=f"tokT{b}_{k}") for k in range(NCH)]
        for b in range(BPC)
    ]
    labT = [
        [const_pool.tile([128, CLS], F32, tag=f"labT{b}_{k}", name=f"labT{b}_{k}") for k in range(NCH)]
        for b in range(BPC)
    ]
    acc = [const_pool.tile([128, NCH, IMG], F32, tag=f"acc{b}", name=f"acc{b}") for b in range(BPC)]

    # ---------------- prep phase ----------------
    with (
        tc.tile_pool(name="prep_sb", bufs=4) as prep_sb,
        tc.tile_pool(name="prep_ps", bufs=4, space="PSUM") as prep_ps,
    ):
        # W3T[kd][:, ke*128:+128] = W3[ke-chunk, kd-chunk].T
        for ke in range(NCH):
            row = prep_sb.tile([128, DIM], F32, tag="w3row")
            nc.sync.dma_start(out=row[:], in_=w3_d[ke * 128 : (ke + 1) * 128, :])
            for kd in range(NCH):
                ps = prep_ps.tile([128, 128], F32, tag="tr")
                nc.tensor.transpose(
                    ps[:], row[:, kd * 128 : (kd + 1) * 128], ident[:]
                )
                nc.scalar.copy(w3t[kd][:, ke * 128 : (ke + 1) * 128], ps[:])

        # W1T[wc][0:wsz, kd*128:+128] = W1[kd-chunk, w-chunk].T
        for kd in range(NCH):
            row = prep_sb.tile([128, WDIM], F32, tag="w1row")
            nc.sync.dma_start(out=row[:], in_=w1_d[kd * 128 : (kd + 1) * 128, :])
            for wc, (woff, wsz) in enumerate(WCH):
                ps = prep_ps.tile([128, 128], F32, tag="tr")
                nc.tensor.transpose(
                    ps[:wsz, :], row[:, woff : woff + wsz], ident[:]
                )
                nc.scalar.copy(w1t[wc][:wsz, kd * 128 : (kd + 1) * 128], ps[:wsz, :])

        for b in range(BPC):
            # tokenT
            for sc, (soff, ssz) in enumerate(SCH):
                ts = prep_sb.tile([128, DIM], F32, tag="tokrow")
                nc.sync.dma_start(
                    out=ts[:ssz, :], in_=token_d[b, soff : soff + ssz, :]
                )
                for kd in range(NCH):
                    ps = prep_ps.tile([128, 128], F32, tag="tr")
                    nc.tensor.transpose(
                        ps[:, :ssz],
                        ts[:ssz, kd * 128 : (kd + 1) * 128],
                        ident[:ssz, :ssz],
                    )
                    nc.scalar.copy(tokT[b][kd][:, soff : soff + ssz], ps[:, :ssz])

            # lgT then labT = (lg @ W1.T).T
            lgrow = prep_sb.tile([128, WDIM], F32, tag="lgrow")
            nc.sync.dma_start(out=lgrow[:CLS, :], in_=lg_d[b, :, :])
            lgt = []
            for wc, (woff, wsz) in enumerate(WCH):
                ps = prep_ps.tile([128, 128], F32, tag="tr")
                nc.tensor.transpose(
                    ps[:wsz, :CLS],
                    lgrow[:CLS, woff : woff + wsz],
                    ident[:CLS, :CLS],
                )
                t = prep_sb.tile([128, CLS], F32, tag=f"lgt{wc}")
                nc.scalar.copy(t[:wsz, :], ps[:wsz, :CLS])
                lgt.append(t)
            for kd in range(NCH):
                ps = prep_ps.tile([128, CLS], F32, tag="lab")
                for wc, (woff, wsz) in enumerate(WCH):
                    nc.tensor.matmul(
                        ps[:],
                        w1t[wc][:wsz, kd * 128 : (kd + 1) * 128],
                        lgt[wc][:wsz, :],
                        start=(wc == 0),
                        stop=(wc == len(WCH) - 1),
                    )
                nc.scalar.copy(labT[b][kd][:], ps[:])

            nc.vector.memset(acc[b][:], 0.0)

    # ---------------- main loop ----------------
    with (
        tc.tile_pool(name="stage", bufs=2) as stage_pool,
        tc.tile_pool(name="prod", bufs=2) as prod_pool,
        tc.tile_pool(name="epool", bufs=2) as e_pool,
        tc.tile_pool(name="zpool", bufs=3) as z_pool,
        tc.tile_pool(name="feat", bufs=2, space="PSUM") as feat_pool,
    ):
        for b in range(BPC):
            for p in range(CLS // 2):
                cA, cB = 2 * p, 2 * p + 1
                stage = stage_pool.tile([128, NCH, 2 * IMG], F32, tag="stage")
                for m in range(NCH):
                    nc.vector.tensor_scalar_mul(
                        stage[:, m, 0:IMG], tokT[b][m][:], labT[b][m][:, cA : cA + 1]
                    )
                    nc.vector.tensor_scalar_mul(
                        stage[:, m, IMG : 2 * IMG],
                        tokT[b][m][:],
                        labT[b][m][:, cB : cB + 1],
                    )
                prod = prod_pool.tile([128, NCH, 2 * IMG], mm_dt, tag="prod")
                for m in range(NCH):
                    nc.scalar.activation(prod[:, m, :], stage[:, m, :], TS.Tanh)

                feat = feat_pool.tile([128, NCH, 512], F32, tag="feat")
                for m in range(NCH):
                    for k in range(NCH):
                        nc.tensor.matmul(
                            feat[:, m, 0 : 2 * IMG],
                            w3t[k][:, m * 128 : (m + 1) * 128],
                            prod[:, k, :],
                            start=(k == 0),
                            stop=(k == NCH - 1),
                        )

                E = e_pool.tile([128, NCH, 2 * IMG], F32, tag="E")
                Z = z_pool.tile([128, 8], F32, tag="Z")
                for m in range(NCH):
                    nc.scalar.activation(
                        E[:, m, 0:IMG],
                        feat[:, m, 0:IMG],
                        TS.Exp,
                        accum_out=Z[:, 2 * m : 2 * m + 1],
                    )
                    nc.scalar.activation(
                        E[:, m, IMG : 2 * IMG],
                        feat[:, m, IMG : 2 * IMG],
                        TS.Exp,
                        accum_out=Z[:, 2 * m + 1 : 2 * m + 2],
                    )
                R = z_pool.tile([128, 8], F32, tag="R")
                nc.vector.reciprocal(R[:], Z[:])
                for m in range(NCH):
                    nc.vector.scalar_tensor_tensor(
                        acc[b][:, m, :],
                        E[:, m, 0:IMG],
                        R[:, 2 * m : 2 * m + 1],
                        acc[b][:, m, :],
                        AO.mult,
                        AO.add,
                    )
                    nc.vector.scalar_tensor_tensor(
                        acc[b][:, m, :],
                        E[:, m, IMG : 2 * IMG],
                        R[:, 2 * m + 1 : 2 * m + 2],
                        acc[b][:, m, :],
                        AO.mult,
                        AO.add,
                    )

    # ---------------- final phase ----------------
    with (
        tc.tile_pool(name="fin_sb", bufs=2) as fin_sb,
        tc.tile_pool(name="fin_ps", bufs=4, space="PSUM") as fin_ps,
    ):
        for b in range(BPC):
            outT = fin_sb.tile([128, NCH, IMG], F32, tag="outT")
            for m in range(NCH):
                nc.vector.tensor_tensor(
                    outT[:, m, :],
                    tokT[b][m][:],
                    acc[b][:, m, :],
                    mybir.AluOpType.mult,
                )
            for sc, (soff, ssz) in enumerate(SCH):
                outs = fin_sb.tile([128, DIM], F32, tag=f"outs{sc}")
                for m in range(NCH):
                    ps = fin_ps.tile([128, 128], F32, tag="tr")
                    nc.tensor.transpose(
                        ps[:ssz, :], outT[:, m, soff : soff + ssz], ident[:]
                    )
                    nc.scalar.copy(outs[:ssz, m * 128 : (m + 1) * 128], ps[:ssz, :])
                nc.sync.dma_start(
                    out=out_d[b, soff : soff + ssz, :], in_=outs[:ssz, :]
                )


_NC_CACHE = {}


def _get_nc(mm_dt=MM_DT):
    key = str(mm_dt)
    if key not in _NC_CACHE:
        _NC_CACHE[key] = build_kernel(mm_dt)
    return _NC_CACHE[key]


def run(inputs, trace=False, mm_dt=MM_DT):
    token = np.ascontiguousarray(np.asarray(inputs["tokenFeaturemap"], np.float32))
    lg = np.ascontiguousarray(np.asarray(inputs["labelGraphfeatures"], np.float32))
    w1 = np.ascontiguousarray(np.asarray(inputs["W1"], np.float32))
    w3 = np.ascontiguousarray(np.asarray(inputs["W3"], np.float32))
    nc = _get_nc(mm_dt)
    in_maps = [
        {
            "token": token[i * BPC : (i + 1) * BPC],
            "lg": lg[i * BPC : (i + 1) * BPC],
            "W1": w1,
            "W3": w3,
        }
        for i in range(N_CORES)
    ]
    res = run_bass_kernel_spmd(nc, in_maps, list(range(N_CORES)), trace=trace)
    out = np.concatenate([res.results[i]["out"] for i in range(N_CORES)], axis=0)
    return out, res


def kernel(**inputs) -> np.ndarray:
    out, _ = run(inputs)
    return out


# revision 9
# speedup vs baseline: 1.8602x; 1.8602x over previous
import os
import sys

for _p in ("/opt/trn_rl_repo", "/root/.axon_site/_ro/trn_rl_repo"):
    if os.path.isdir(_p) and _p not in sys.path:
        sys.path.append(_p)

import numpy as np
import concourse.bass as bass
import concourse.mybir as mybir
import concourse.tile as tile
from concourse import masks
from concourse.bass_utils import run_bass_kernel_spmd

# Problem shapes (hardcoded per contract)
BS, IMG, CLS, DIM, WDIM = 16, 196, 80, 512, 300
N_CORES = 8
BPC = BS // N_CORES  # batches per core
NCH = DIM // 128  # 4 d-chunks of 128
SCH = [(0, 128), (128, 68)]  # s-chunks (offset, size) for IMG=196
WCH = [(0, 128), (128, 128), (256, 44)]  # w-chunks for WDIM=300

F32 = mybir.dt.float32
# dtype used for the fc3 matmul operands (W3T weights and prodT moving data)
MM_DT = mybir.dt.float32


def split_multi_waits(nc):
    """This walrus build accepts a single sync-wait per instruction on the
    CTRL encodings; split extra waits into single-wait NoOps on the same
    engine immediately before the instruction."""
    k = 0
    for f in nc.m.functions:
        for bb in f.blocks:
            il = bb.instructions
            i = 0
            while i < len(il):
                ins = il[i]
                si = ins.sync_info
                if si is not None and len(si.on_wait) > 1:
                    waits = list(si.on_wait)
                    for w in waits[:-1]:
                        nop = mybir.InstNoOp(name=f"waitsplit_{k}", ins=[], outs=[])
                        k += 1
                        nop.engine = ins.engine
                        nop.sync_info = mybir.SyncInfo(on_wait=[w], on_update=[])
                        il.insert(i, nop)
                        i += 1
                    ins.sync_info = mybir.SyncInfo(
                        on_wait=[waits[-1]], on_update=list(si.on_update)
                    )
                i += 1
    return k


def build_kernel(mm_dt=MM_DT, variant="v2", zdve_frac=0.64):
    nc = bass.Bass("TRN2", target_bir_lowering=False, debug=False, num_devices=N_CORES)
    token_d = nc.dram_tensor("token", [BPC, IMG, DIM], F32, kind="ExternalInput").ap()
    lg_d = nc.dram_tensor("lg", [BPC, CLS, WDIM], F32, kind="ExternalInput").ap()
    w1_d = nc.dram_tensor("W1", [DIM, WDIM], F32, kind="ExternalInput").ap()
    w3_d = nc.dram_tensor("W3", [DIM, DIM], F32, kind="ExternalInput").ap()
    out_d = nc.dram_tensor("out", [BPC, IMG, DIM], F32, kind="ExternalOutput").ap()

    with tile.TileContext(nc) as tc:
        if variant == "v1":
            emit_body(tc, nc, token_d, lg_d, w1_d, w3_d, out_d, mm_dt)
        else:
            emit_body_v2(tc, nc, token_d, lg_d, w1_d, w3_d, out_d, zdve_frac)
    split_multi_waits(nc)
    return nc


def emit_body(tc, nc, token_d, lg_d, w1_d, w3_d, out_d, mm_dt):
    TS = mybir.ActivationFunctionType
    AO = mybir.AluOpType

    with tc.tile_pool(name="const", bufs=1) as const_pool:
        _emit_body_inner(tc, nc, token_d, lg_d, w1_d, w3_d, out_d, mm_dt, const_pool)


def _emit_body_inner(tc, nc, token_d, lg_d, w1_d, w3_d, out_d, mm_dt, const_pool):
    TS = mybir.ActivationFunctionType
    AO = mybir.AluOpType

    ident = const_pool.tile([128, 128], F32)
    masks.make_identity(nc, ident[:])

    # Persistent SBUF tensors
    w3t = [const_pool.tile([128, DIM], mm_dt, tag=f"w3t{k}", name=f"w3t{k}") for k in range(NCH)]
    w1t = [const_pool.tile([128, DIM], F32, tag=f"w1t{w}", name=f"w1t{w}") for w in range(len(WCH))]
    tokT = [
        [const_pool.tile([128, IMG], F32, tag=f"tokT{b}_{k}", name=f"tokT{b}_{k}") for k in range(NCH)]
        for b in range(BPC)
    ]
    labT = [
        [const_pool.tile([128, CLS], F32, tag=f"labT{b}_{k}", name=f"labT{b}_{k}") for k in range(NCH)]
        for b in range(BPC)
    ]
    acc = [const_pool.tile([128, NCH, IMG], F32, tag=f"acc{b}", name=f"acc{b}") for b in range(BPC)]

    # ---------------- prep phase ----------------
    with (
        tc.tile_pool(name="prep_sb", bufs=4) as prep_sb,
        tc.tile_pool(name="prep_ps", bufs=4, space="PSUM") as prep_ps,
    ):
        # W3T[kd][:, ke*128:+128] = W3[ke-chunk, kd-chunk].T
        for ke in range(NCH):
            row = prep_sb.tile([128, DIM], F32, tag="w3row")
            nc.sync.dma_start(out=row[:], in_=w3_d[ke * 128 : (ke + 1) * 128, :])
            for kd in range(NCH):
                ps = prep_ps.tile([128, 128], F32, tag="tr")
                nc.tensor.transpose(
                    ps[:], row[:, kd * 128 : (kd + 1) * 128], ident[:]
                )
                nc.scalar.copy(w3t[kd][:, ke * 128 : (ke + 1) * 128], ps[:])

        # W1T[wc][0:wsz, kd*128:+128] = W1[kd-chunk, w-chunk].T
        for kd in range(NCH):
            row = prep_sb.tile([128, WDIM], F32, tag="w1row")
            nc.sync.dma_start(out=row[:], in_=w1_d[kd * 128 : (kd + 1) * 128, :])
            for wc, (woff, wsz) in enumerate(WCH):
                ps = prep_ps.tile([128, 128], F32, tag="tr")
                nc.tensor.transpose(
                    ps[:wsz, :], row[:, woff : woff + wsz], ident[:]
                )
                nc.scalar.copy(w1t[wc][:wsz, kd * 128 : (kd + 1) * 128], ps[:wsz, :])

        for b in range(BPC):
            # tokenT
            for sc, (soff, ssz) in enumerate(SCH):
                ts = prep_sb.tile([128, DIM], F32, tag="tokrow")
                nc.sync.dma_start(
                    out=ts[:ssz, :], in_=token_d[b, soff : soff + ssz, :]
                )
                for kd in range(NCH):
                    ps = prep_ps.tile([128, 128], F32, tag="tr")
                    nc.tensor.transpose(
                        ps[:, :ssz],
                        ts[:ssz, kd * 128 : (kd + 1) * 128],
                        ident[:ssz, :ssz],
                    )
                    nc.scalar.copy(tokT[b][kd][:, soff : soff + ssz], ps[:, :ssz])

            # lgT then labT = (lg @ W1.T).T
            lgrow = prep_sb.tile([128, WDIM], F32, tag="lgrow")
            nc.sync.dma_start(out=lgrow[:CLS, :], in_=lg_d[b, :, :])
            lgt = []
            for wc, (woff, wsz) in enumerate(WCH):
                ps = prep_ps.tile([128, 128], F32, tag="tr")
                nc.tensor.transpose(
                    ps[:wsz, :CLS],
                    lgrow[:CLS, woff : woff + wsz],
                    ident[:CLS, :CLS],
                )
                t = prep_sb.tile([128, CLS], F32, tag=f"lgt{wc}")
                nc.scalar.copy(t[:wsz, :], ps[:wsz, :CLS])
                lgt.append(t)
            for kd in range(NCH):
                ps = prep_ps.tile([128, CLS], F32, tag="lab")
                for wc, (woff, wsz) in enumerate(WCH):
                    nc.tensor.matmul(
                        ps[:],
                        w1t[wc][:wsz, kd * 128 : (kd + 1) * 128],
                        lgt[wc][:wsz, :],
                        start=(wc == 0),
                        stop=(wc == len(WCH) - 1),
                    )
                nc.scalar.copy(labT[b][kd][:], ps[:])

            nc.vector.memset(acc[b][:], 0.0)

    # ---------------- main loop ----------------
    with (
        tc.tile_pool(name="stage", bufs=2) as stage_pool,
        tc.tile_pool(name="prod", bufs=2) as prod_pool,
        tc.tile_pool(name="epool", bufs=2) as e_pool,
        tc.tile_pool(name="zpool", bufs=3) as z_pool,
        tc.tile_pool(name="feat", bufs=2, space="PSUM") as feat_pool,
    ):
        for b in range(BPC):
            for p in range(CLS // 2):
                cA, cB = 2 * p, 2 * p + 1
                stage = stage_pool.tile([128, NCH, 2 * IMG], F32, tag="stage")
                for m in range(NCH):
                    nc.vector.tensor_scalar_mul(
                        stage[:, m, 0:IMG], tokT[b][m][:], labT[b][m][:, cA : cA + 1]
                    )
                    nc.vector.tensor_scalar_mul(
                        stage[:, m, IMG : 2 * IMG],
                        tokT[b][m][:],
                        labT[b][m][:, cB : cB + 1],
                    )
                prod = prod_pool.tile([128, NCH, 2 * IMG], mm_dt, tag="prod")
                for m in range(NCH):
                    nc.scalar.activation(prod[:, m, :], stage[:, m, :], TS.Tanh)

                feat = feat_pool.tile([128, NCH, 512], F32, tag="feat")
                for m in range(NCH):
                    for k in range(NCH):
                        nc.tensor.matmul(
                            feat[:, m, 0 : 2 * IMG],
                            w3t[k][:, m * 128 : (m + 1) * 128],
                            prod[:, k, :],
                            start=(k == 0),
                            stop=(k == NCH - 1),
                        )

                E = e_pool.tile([128, NCH, 2 * IMG], F32, tag="E")
                Z = z_pool.tile([128, 8], F32, tag="Z")
                for m in range(NCH):
                    nc.scalar.activation(
                        E[:, m, 0:IMG],
                        feat[:, m, 0:IMG],
                        TS.Exp,
                        accum_out=Z[:, 2 * m : 2 * m + 1],
                    )
                    nc.scalar.activation(
                        E[:, m, IMG : 2 * IMG],
                        feat[:, m, IMG : 2 * IMG],
                        TS.Exp,
                        accum_out=Z[:, 2 * m + 1 : 2 * m + 2],
                    )
                R = z_pool.tile([128, 8], F32, tag="R")
                nc.vector.reciprocal(R[:], Z[:])
                for m in range(NCH):
                    nc.vector.scalar_tensor_tensor(
                        acc[b][:, m, :],
                        E[:, m, 0:IMG],
                        R[:, 2 * m : 2 * m + 1],
                        acc[b][:, m, :],
                        AO.mult,
                        AO.add,
                    )
                    nc.vector.scalar_tensor_tensor(
                        acc[b][:, m, :],
                        E[:, m, IMG : 2 * IMG],
                        R[:, 2 * m + 1 : 2 * m + 2],
                        acc[b][:, m, :],
                        AO.mult,
                        AO.add,
                    )

    # ---------------- final phase ----------------
    with (
        tc.tile_pool(name="fin_sb", bufs=2) as fin_sb,
        tc.tile_pool(name="fin_ps", bufs=4, space="PSUM") as fin_ps,
    ):
        for b in range(BPC):
            outT = fin_sb.tile([128, NCH, IMG], F32, tag="outT")
            for m in range(NCH):
                nc.vector.tensor_tensor(
                    outT[:, m, :],
                    tokT[b][m][:],
                    acc[b][:, m, :],
                    mybir.AluOpType.mult,
                )
            for sc, (soff, ssz) in enumerate(SCH):
                outs = fin_sb.tile([128, DIM], F32, tag=f"outs{sc}")
                for m in range(NCH):
                    ps = fin_ps.tile([128, 128], F32, tag="tr")
                    nc.tensor.transpose(
                        ps[:ssz, :], outT[:, m, soff : soff + ssz], ident[:]
                    )
                    nc.scalar.copy(outs[:ssz, m * 128 : (m + 1) * 128], ps[:ssz, :])
                nc.sync.dma_start(
                    out=out_d[b, soff : soff + ssz, :], in_=outs[:ssz, :]
                )


def emit_body_v2(tc, nc, token_d, lg_d, w1_d, w3_d, out_d, zdve_frac=0.64):
    with tc.tile_pool(name="const", bufs=1) as const_pool:
        _emit_v2_inner(tc, nc, token_d, lg_d, w1_d, w3_d, out_d, const_pool, zdve_frac)


def _emit_v2_inner(tc, nc, token_d, lg_d, w1_d, w3_d, out_d, cp, zdve_frac):
    TS = mybir.ActivationFunctionType
    AO = mybir.AluOpType
    BF16 = mybir.dt.bfloat16
    X = mybir.AxisListType.X

    ident = cp.tile([128, 128], F32)
    masks.make_identity(nc, ident[:])
    ident_bf = cp.tile([128, 128], BF16)
    masks.make_identity(nc, ident_bf[:])

    w3t = [cp.tile([128, DIM], BF16, tag=f"w3t{k}", name=f"w3t{k}") for k in range(NCH)]
    w1t = [cp.tile([128, DIM], F32, tag=f"w1t{w}", name=f"w1t{w}") for w in range(len(WCH))]
    tokT = [
        [cp.tile([128, IMG], F32, tag=f"tokT{b}_{k}", name=f"tokT{b}_{k}") for k in range(NCH)]
        for b in range(BPC)
    ]
    tokB = [
        [cp.tile([128, IMG], BF16, tag=f"tokB{b}_{k}", name=f"tokB{b}_{k}") for k in range(NCH)]
        for b in range(BPC)
    ]
    labT = [
        [cp.tile([128, CLS], F32, tag=f"labT{b}_{k}", name=f"labT{b}_{k}") for k in range(NCH)]
        for b in range(BPC)
    ]

    # ---------------- prep ----------------
    with (
        tc.tile_pool(name="prep_sb", bufs=4) as prep_sb,
        tc.tile_pool(name="prep_ps", bufs=4, space="PSUM") as prep_ps,
    ):
        for ke in range(NCH):
            row = prep_sb.tile([128, DIM], F32, tag="w3row")
            nc.sync.dma_start(out=row[:], in_=w3_d[ke * 128 : (ke + 1) * 128, :])
            for kd in range(NCH):
                ps = prep_ps.tile([128, 128], F32, tag="tr")
                nc.tensor.transpose(ps[:], row[:, kd * 128 : (kd + 1) * 128], ident[:])
                nc.scalar.copy(w3t[kd][:, ke * 128 : (ke + 1) * 128], ps[:])

        for kd in range(NCH):
            row = prep_sb.tile([128, WDIM], F32, tag="w1row")
            nc.sync.dma_start(out=row[:], in_=w1_d[kd * 128 : (kd + 1) * 128, :])
            for wc, (woff, wsz) in enumerate(WCH):
                ps = prep_ps.tile([128, 128], F32, tag="tr")
                nc.tensor.transpose(ps[:wsz, :], row[:, woff : woff + wsz], ident[:])
                nc.scalar.copy(w1t[wc][:wsz, kd * 128 : (kd + 1) * 128], ps[:wsz, :])

        for b in range(BPC):
            for sc, (soff, ssz) in enumerate(SCH):
                ts_ = prep_sb.tile([128, DIM], F32, tag="tokrow")
                nc.sync.dma_start(out=ts_[:ssz, :], in_=token_d[b, soff : soff + ssz, :])
                for kd in range(NCH):
                    ps = prep_ps.tile([128, 128], F32, tag="tr")
                    nc.tensor.transpose(
                        ps[:, :ssz],
                        ts_[:ssz, kd * 128 : (kd + 1) * 128],
                        ident[:ssz, :ssz],
                    )
                    nc.scalar.copy(tokT[b][kd][:, soff : soff + ssz], ps[:, :ssz])
            for kd in range(NCH):
                nc.vector.tensor_copy(tokB[b][kd][:], tokT[b][kd][:])

            lgrow = prep_sb.tile([128, WDIM], F32, tag="lgrow")
            nc.sync.dma_start(out=lgrow[:CLS, :], in_=lg_d[b, :, :])
            lgt = []
            for wc, (woff, wsz) in enumerate(WCH):
                ps = prep_ps.tile([128, 128], F32, tag="tr")
                nc.tensor.transpose(
                    ps[:wsz, :CLS], lgrow[:CLS, woff : woff + wsz], ident[:CLS, :CLS]
                )
                t = prep_sb.tile([128, CLS], F32, tag=f"lgt{wc}")
                nc.scalar.copy(t[:wsz, :], ps[:wsz, :CLS])
                lgt.append(t)
            for kd in range(NCH):
                ps = prep_ps.tile([128, CLS], F32, tag="lab")
                for wc, (woff, wsz) in enumerate(WCH):
                    nc.tensor.matmul(
                        ps[:],
                        w1t[wc][:wsz, kd * 128 : (kd + 1) * 128],
                        lgt[wc][:wsz, :],
                        start=(wc == 0),
                        stop=(wc == len(WCH) - 1),
                    )
                nc.scalar.copy(labT[b][kd][:], ps[:])

    # ---------------- main ----------------
    NPAIR = CLS // 2  # 40 pairs per batch
    OCT = 4  # pairs per octet group (8 classes)
    with tc.tile_pool(name="accps", bufs=1, space="PSUM") as acc_pool, tc.tile_pool(
        name="fin_sb", bufs=2
    ) as fin_sb:
        for b in range(BPC):
            acc_ps = acc_pool.tile([128, NCH, 512], F32, tag="acc", name=f"accps{b}")
            with (
                tc.tile_pool(name="stage", bufs=2) as stage_pool,
                tc.tile_pool(name="prod", bufs=2) as prod_pool,
                tc.tile_pool(name="epool", bufs=2) as e_pool,
                tc.tile_pool(name="zpool", bufs=3) as z_pool,
                tc.tile_pool(name="feat", bufs=2, space="PSUM") as feat_pool,
            ):
                for g in range(NPAIR // OCT):  # octet groups
                    stage = stage_pool.tile([128, NCH, OCT * 2 * IMG], BF16, tag="stage")
                    prod = prod_pool.tile([128, NCH, OCT * 2 * IMG], BF16, tag="prod")
                    for m in range(NCH):
                        for j in range(2 * OCT):
                            c = g * 2 * OCT + j
                            nc.vector.tensor_scalar_mul(
                                stage[:, m, j * IMG : (j + 1) * IMG],
                                tokB[b][m][:],
                                labT[b][m][:, c : c + 1],
                            )
                        nc.scalar.activation(prod[:, m, :], stage[:, m, :], TS.Tanh)
                    for pj in range(OCT):
                        p = g * OCT + pj
                        kf = int(round(zdve_frac * NPAIR))
                        z_dve = ((p + 1) * kf) // NPAIR > (p * kf) // NPAIR
                        E = e_pool.tile([128, NCH, 2 * IMG], BF16, tag="E", name=f"E{b}_{p}")
                        Z = z_pool.tile([128, 8], F32, tag="Z", name=f"Z{b}_{p}")
                        R = z_pool.tile([128, 8], F32, tag="R", name=f"R{b}_{p}")
                        for half in range(2):
                            feat = feat_pool.tile(
                                [128, 2, 512], F32, tag="feat", name=f"feat{b}_{p}_{half}"
                            )
                            for mi in range(2):
                                m = half * 2 + mi
                                for k in range(NCH):
                                    nc.tensor.matmul(
                                        feat[:, mi, 0 : 2 * IMG],
                                        w3t[k][:, m * 128 : (m + 1) * 128],
                                        prod[:, k, pj * 2 * IMG : (pj + 1) * 2 * IMG],
                                        start=(k == 0),
                                        stop=(k == NCH - 1),
                                    )
                            if z_dve:
                                nc.scalar.activation(
                                    E[:, half * 2 : half * 2 + 2, :],
                                    feat[:, :, 0 : 2 * IMG],
                                    TS.Exp,
                                )
                            else:
                                for mi in range(2):
                                    m = half * 2 + mi
                                    for j in range(2):
                                        nc.scalar.activation(
                                            E[:, m, j * IMG : (j + 1) * IMG],
                                            feat[:, mi, j * IMG : (j + 1) * IMG],
                                            TS.Exp,
                                            accum_out=Z[:, 2 * m + j : 2 * m + j + 1],
                                        )
                        if z_dve:
                            nc.vector.tensor_reduce(
                                Z[:],
                                E[:].rearrange("p m (j s) -> p m j s", j=2),
                                axis=X,
                                op=AO.add,
                            )
                        nc.vector.reciprocal(R[:], Z[:])
                        # scale E in place, then accumulate via identity matmul
                        for m in range(NCH):
                            for j in range(2):
                                nc.vector.tensor_scalar_mul(
                                    E[:, m, j * IMG : (j + 1) * IMG],
                                    E[:, m, j * IMG : (j + 1) * IMG],
                                    R[:, 2 * m + j : 2 * m + j + 1],
                                )
                            nc.tensor.matmul(
                                acc_ps[:, m, 0 : 2 * IMG],
                                ident_bf[:],
                                E[:, m, :],
                                start=(p == 0),
                                stop=(p == NPAIR - 1),
                                skip_group_check=True,
                            )

            # ---------------- final for this batch ----------------
            with tc.tile_pool(name="fin_ps", bufs=4, space="PSUM") as fin_ps:
                outT = fin_sb.tile([128, NCH, IMG], F32, tag="outT", name=f"outT{b}")
                tmp2 = fin_sb.tile([128, NCH, 2 * IMG], F32, tag="tmpacc", name=f"tmpacc{b}")
                tmp = fin_sb.tile([128, NCH, IMG], F32, tag="tmpsum", name=f"tmpsum{b}")
                for m in range(NCH):
                    nc.scalar.copy(tmp2[:, m, :], acc_ps[:, m, 0 : 2 * IMG])
                    nc.vector.tensor_tensor(
                        tmp[:, m, :], tmp2[:, m, 0:IMG], tmp2[:, m, IMG : 2 * IMG], AO.add
                    )
                    nc.vector.tensor_tensor(
                        outT[:, m, :], tokT[b][m][:], tmp[:, m, :], AO.mult
                    )
                for sc, (soff, ssz) in enumerate(SCH):
                    outs = fin_sb.tile([128, DIM], F32, tag=f"outs{sc}", name=f"outs{b}_{sc}")
                    for m in range(NCH):
                        ps = fin_ps.tile([128, 128], F32, tag="tr")
                        nc.tensor.transpose(
                            ps[:ssz, :], outT[:, m, soff : soff + ssz], ident[:]
                        )
                        nc.scalar.copy(outs[:ssz, m * 128 : (m + 1) * 128], ps[:ssz, :])
                    nc.sync.dma_start(out=out_d[b, soff : soff + ssz, :], in_=outs[:ssz, :])


_NC_CACHE = {}


def _get_nc(mm_dt=MM_DT, variant="v2", zdve_frac=0.64):
    key = (str(mm_dt), variant, zdve_frac)
    if key not in _NC_CACHE:
        _NC_CACHE[key] = build_kernel(mm_dt, variant, zdve_frac)
    return _NC_CACHE[key]


def run(inputs, trace=False, mm_dt=MM_DT, variant="v2", zdve_frac=0.64):
    token = np.ascontiguousarray(np.asarray(inputs["tokenFeaturemap"], np.float32))
    lg = np.ascontiguousarray(np.asarray(inputs["labelGraphfeatures"], np.float32))
    w1 = np.ascontiguousarray(np.asarray(inputs["W1"], np.float32))
    w3 = np.ascontiguousarray(np.asarray(inputs["W3"], np.float32))
    nc = _get_nc(mm_dt, variant, zdve_frac)
    in_maps = [
        {
            "token": token[i * BPC : (i + 1) * BPC],
            "lg": lg[i * BPC : (i + 1) * BPC],
            "W1": w1,
            "W3": w3,
        }
        for i in range(N_CORES)
    ]
    res = run_bass_kernel_spmd(nc, in_maps, list(range(N_CORES)), trace=trace)
    out = np.concatenate([res.results[i]["out"] for i in range(N_CORES)], axis=0)
    return out, res


def kernel(**inputs) -> np.ndarray:
    out, _ = run(inputs)
    return out


# revision 11
# speedup vs baseline: 1.9640x; 1.0558x over previous
import os
import sys

for _p in ("/opt/trn_rl_repo", "/root/.axon_site/_ro/trn_rl_repo"):
    if os.path.isdir(_p) and _p not in sys.path:
        sys.path.append(_p)

import numpy as np
import concourse.bass as bass
import concourse.mybir as mybir
import concourse.tile as tile
from concourse import masks
from concourse.bass_utils import run_bass_kernel_spmd

# Problem shapes (hardcoded per contract)
BS, IMG, CLS, DIM, WDIM = 16, 196, 80, 512, 300
N_CORES = 8
BPC = BS // N_CORES  # batches per core
NCH = DIM // 128  # 4 d-chunks of 128
SCH = [(0, 128), (128, 68)]  # s-chunks (offset, size) for IMG=196
WCH = [(0, 128), (128, 128), (256, 44)]  # w-chunks for WDIM=300

F32 = mybir.dt.float32
# dtype used for the fc3 matmul operands (W3T weights and prodT moving data)
MM_DT = mybir.dt.float32


def split_multi_waits(nc):
    """This walrus build accepts a single sync-wait per instruction on the
    CTRL encodings; split extra waits into single-wait NoOps on the same
    engine immediately before the instruction."""
    k = 0
    for f in nc.m.functions:
        for bb in f.blocks:
            il = bb.instructions
            i = 0
            while i < len(il):
                ins = il[i]
                si = ins.sync_info
                if si is not None and len(si.on_wait) > 1:
                    waits = list(si.on_wait)
                    for w in waits[:-1]:
                        nop = mybir.InstNoOp(name=f"waitsplit_{k}", ins=[], outs=[])
                        k += 1
                        nop.engine = ins.engine
                        nop.sync_info = mybir.SyncInfo(on_wait=[w], on_update=[])
                        il.insert(i, nop)
                        i += 1
                    ins.sync_info = mybir.SyncInfo(
                        on_wait=[waits[-1]], on_update=list(si.on_update)
                    )
                i += 1
    return k


def build_kernel(mm_dt=MM_DT, variant="v2", zdve_frac=0.64):
    nc = bass.Bass("TRN2", target_bir_lowering=False, debug=False, num_devices=N_CORES)
    token_d = nc.dram_tensor("token", [BPC, IMG, DIM], F32, kind="ExternalInput").ap()
    lg_d = nc.dram_tensor("lg", [BPC, CLS, WDIM], F32, kind="ExternalInput").ap()
    w1_d = nc.dram_tensor("W1", [DIM, WDIM], F32, kind="ExternalInput").ap()
    w3_d = nc.dram_tensor("W3", [DIM, DIM], F32, kind="ExternalInput").ap()
    out_d = nc.dram_tensor("out", [BPC, IMG, DIM], F32, kind="ExternalOutput").ap()

    with tile.TileContext(nc) as tc:
        if variant == "v1":
            emit_body(tc, nc, token_d, lg_d, w1_d, w3_d, out_d, mm_dt)
        else:
            emit_body_v2(tc, nc, token_d, lg_d, w1_d, w3_d, out_d, zdve_frac)
    split_multi_waits(nc)
    return nc


def emit_body(tc, nc, token_d, lg_d, w1_d, w3_d, out_d, mm_dt):
    TS = mybir.ActivationFunctionType
    AO = mybir.AluOpType

    with tc.tile_pool(name="const", bufs=1) as const_pool:
        _emit_body_inner(tc, nc, token_d, lg_d, w1_d, w3_d, out_d, mm_dt, const_pool)


def _emit_body_inner(tc, nc, token_d, lg_d, w1_d, w3_d, out_d, mm_dt, const_pool):
    TS = mybir.ActivationFunctionType
    AO = mybir.AluOpType

    ident = const_pool.tile([128, 128], F32)
    masks.make_identity(nc, ident[:])

    # Persistent SBUF tensors
    w3t = [const_pool.tile([128, DIM], mm_dt, tag=f"w3t{k}", name=f"w3t{k}") for k in range(NCH)]
    w1t = [const_pool.tile([128, DIM], F32, tag=f"w1t{w}", name=f"w1t{w}") for w in range(len(WCH))]
    tokT = [
        [const_pool.tile([128, IMG], F32, tag=f"tokT{b}_{k}", name=f"tokT{b}_{k}") for k in range(NCH)]
        for b in range(BPC)
    ]
    labT = [
        [const_pool.tile([128, CLS], F32, tag=f"labT{b}_{k}", name=f"labT{b}_{k}") for k in range(NCH)]
        for b in range(BPC)
    ]
    acc = [const_pool.tile([128, NCH, IMG], F32, tag=f"acc{b}", name=f"acc{b}") for b in range(BPC)]

    # ---------------- prep phase ----------------
    with (
        tc.tile_pool(name="prep_sb", bufs=4) as prep_sb,
        tc.tile_pool(name="prep_ps", bufs=4, space="PSUM") as prep_ps,
    ):
        # W3T[kd][:, ke*128:+128] = W3[ke-chunk, kd-chunk].T
        for ke in range(NCH):
            row = prep_sb.tile([128, DIM], F32, tag="w3row")
            nc.sync.dma_start(out=row[:], in_=w3_d[ke * 128 : (ke + 1) * 128, :])
            for kd in range(NCH):
                ps = prep_ps.tile([128, 128], F32, tag="tr")
                nc.tensor.transpose(
                    ps[:], row[:, kd * 128 : (kd + 1) * 128], ident[:]
                )
                nc.scalar.copy(w3t[kd][:, ke * 128 : (ke + 1) * 128], ps[:])

        # W1T[wc][0:wsz, kd*128:+128] = W1[kd-chunk, w-chunk].T
        for kd in range(NCH):
            row = prep_sb.tile([128, WDIM], F32, tag="w1row")
            nc.sync.dma_start(out=row[:], in_=w1_d[kd * 128 : (kd + 1) * 128, :])
            for wc, (woff, wsz) in enumerate(WCH):
                ps = prep_ps.tile([128, 128], F32, tag="tr")
                nc.tensor.transpose(
                    ps[:wsz, :], row[:, woff : woff + wsz], ident[:]
                )
                nc.scalar.copy(w1t[wc][:wsz, kd * 128 : (kd + 1) * 128], ps[:wsz, :])

        for b in range(BPC):
            # tokenT
            for sc, (soff, ssz) in enumerate(SCH):
                ts = prep_sb.tile([128, DIM], F32, tag="tokrow")
                nc.sync.dma_start(
                    out=ts[:ssz, :], in_=token_d[b, soff : soff + ssz, :]
                )
                for kd in range(NCH):
                    ps = prep_ps.tile([128, 128], F32, tag="tr")
                    nc.tensor.transpose(
                        ps[:, :ssz],
                        ts[:ssz, kd * 128 : (kd + 1) * 128],
                        ident[:ssz, :ssz],
                    )
                    nc.scalar.copy(tokT[b][kd][:, soff : soff + ssz], ps[:, :ssz])

            # lgT then labT = (lg @ W1.T).T
            lgrow = prep_sb.tile([128, WDIM], F32, tag="lgrow")
            nc.sync.dma_start(out=lgrow[:CLS, :], in_=lg_d[b, :, :])
            lgt = []
            for wc, (woff, wsz) in enumerate(WCH):
                ps = prep_ps.tile([128, 128], F32, tag="tr")
                nc.tensor.transpose(
                    ps[:wsz, :CLS],
                    lgrow[:CLS, woff : woff + wsz],
                    ident[:CLS, :CLS],
                )
                t = prep_sb.tile([128, CLS], F32, tag=f"lgt{wc}")
                nc.scalar.copy(t[:wsz, :], ps[:wsz, :CLS])
                lgt.append(t)
            for kd in range(NCH):
                ps = prep_ps.tile([128, CLS], F32, tag="lab")
                for wc, (woff, wsz) in enumerate(WCH):
                    nc.tensor.matmul(
                        ps[:],
                        w1t[wc][:wsz, kd * 128 : (kd + 1) * 128],
                        lgt[wc][:wsz, :],
                        start=(wc == 0),
                        stop=(wc == len(WCH) - 1),
                    )
                nc.scalar.copy(labT[b][kd][:], ps[:])

            nc.vector.memset(acc[b][:], 0.0)

    # ---------------- main loop ----------------
    with (
        tc.tile_pool(name="stage", bufs=2) as stage_pool,
        tc.tile_pool(name="prod", bufs=2) as prod_pool,
        tc.tile_pool(name="epool", bufs=2) as e_pool,
        tc.tile_pool(name="zpool", bufs=3) as z_pool,
        tc.tile_pool(name="feat", bufs=2, space="PSUM") as feat_pool,
    ):
        for b in range(BPC):
            for p in range(CLS // 2):
                cA, cB = 2 * p, 2 * p + 1
                stage = stage_pool.tile([128, NCH, 2 * IMG], F32, tag="stage")
                for m in range(NCH):
                    nc.vector.tensor_scalar_mul(
                        stage[:, m, 0:IMG], tokT[b][m][:], labT[b][m][:, cA : cA + 1]
                    )
                    nc.vector.tensor_scalar_mul(
                        stage[:, m, IMG : 2 * IMG],
                        tokT[b][m][:],
                        labT[b][m][:, cB : cB + 1],
                    )
                prod = prod_pool.tile([128, NCH, 2 * IMG], mm_dt, tag="prod")
                for m in range(NCH):
                    nc.scalar.activation(prod[:, m, :], stage[:, m, :], TS.Tanh)

                feat = feat_pool.tile([128, NCH, 512], F32, tag="feat")
                for m in range(NCH):
                    for k in range(NCH):
                        nc.tensor.matmul(
                            feat[:, m, 0 : 2 * IMG],
                            w3t[k][:, m * 128 : (m + 1) * 128],
                            prod[:, k, :],
                            start=(k == 0),
                            stop=(k == NCH - 1),
                        )

                E = e_pool.tile([128, NCH, 2 * IMG], F32, tag="E")
                Z = z_pool.tile([128, 8], F32, tag="Z")
                for m in range(NCH):
                    nc.scalar.activation(
                        E[:, m, 0:IMG],
                        feat[:, m, 0:IMG],
                        TS.Exp,
                        accum_out=Z[:, 2 * m : 2 * m + 1],
                    )
                    nc.scalar.activation(
                        E[:, m, IMG : 2 * IMG],
                        feat[:, m, IMG : 2 * IMG],
                        TS.Exp,
                        accum_out=Z[:, 2 * m + 1 : 2 * m + 2],
                    )
                R = z_pool.tile([128, 8], F32, tag="R")
                nc.vector.reciprocal(R[:], Z[:])
                for m in range(NCH):
                    nc.vector.scalar_tensor_tensor(
                        acc[b][:, m, :],
                        E[:, m, 0:IMG],
                        R[:, 2 * m : 2 * m + 1],
                        acc[b][:, m, :],
                        AO.mult,
                        AO.add,
                    )
                    nc.vector.scalar_tensor_tensor(
                        acc[b][:, m, :],
                        E[:, m, IMG : 2 * IMG],
                        R[:, 2 * m + 1 : 2 * m + 2],
                        acc[b][:, m, :],
                        AO.mult,
                        AO.add,
                    )

    # ---------------- final phase ----------------
    with (
        tc.tile_pool(name="fin_sb", bufs=2) as fin_sb,
        tc.tile_pool(name="fin_ps", bufs=4, space="PSUM") as fin_ps,
    ):
        for b in range(BPC):
            outT = fin_sb.tile([128, NCH, IMG], F32, tag="outT")
            for m in range(NCH):
                nc.vector.tensor_tensor(
                    outT[:, m, :],
                    tokT[b][m][:],
                    acc[b][:, m, :],
                    mybir.AluOpType.mult,
                )
            for sc, (soff, ssz) in enumerate(SCH):
                outs = fin_sb.tile([128, DIM], F32, tag=f"outs{sc}")
                for m in range(NCH):
                    ps = fin_ps.tile([128, 128], F32, tag="tr")
                    nc.tensor.transpose(
                        ps[:ssz, :], outT[:, m, soff : soff + ssz], ident[:]
                    )
                    nc.scalar.copy(outs[:ssz, m * 128 : (m + 1) * 128], ps[:ssz, :])
                nc.sync.dma_start(
                    out=out_d[b, soff : soff + ssz, :], in_=outs[:ssz, :]
                )


def emit_body_v2(tc, nc, token_d, lg_d, w1_d, w3_d, out_d, zdve_frac=0.64):
    with tc.tile_pool(name="const", bufs=1) as const_pool:
        _emit_v2_inner(tc, nc, token_d, lg_d, w1_d, w3_d, out_d, const_pool, zdve_frac)


def _emit_v2_inner(tc, nc, token_d, lg_d, w1_d, w3_d, out_d, cp, zdve_frac):
    TS = mybir.ActivationFunctionType
    AO = mybir.AluOpType
    BF16 = mybir.dt.bfloat16
    X = mybir.AxisListType.X

    ident = cp.tile([128, 128], F32)
    masks.make_identity(nc, ident[:])
    ident_bf = cp.tile([128, 128], BF16)
    masks.make_identity(nc, ident_bf[:])

    w3t = [cp.tile([128, DIM], BF16, tag=f"w3t{k}", name=f"w3t{k}") for k in range(NCH)]
    w1t = [cp.tile([128, DIM], F32, tag=f"w1t{w}", name=f"w1t{w}") for w in range(len(WCH))]
    tokT = [
        [cp.tile([128, IMG], F32, tag=f"tokT{b}_{k}", name=f"tokT{b}_{k}") for k in range(NCH)]
        for b in range(BPC)
    ]
    tokB = [
        [cp.tile([128, IMG], BF16, tag=f"tokB{b}_{k}", name=f"tokB{b}_{k}") for k in range(NCH)]
        for b in range(BPC)
    ]
    labT = [
        [cp.tile([128, CLS], F32, tag=f"labT{b}_{k}", name=f"labT{b}_{k}") for k in range(NCH)]
        for b in range(BPC)
    ]

    # ---------------- prep ----------------
    with (
        tc.tile_pool(name="prep_sb", bufs=4) as prep_sb,
        tc.tile_pool(name="prep_ps", bufs=4, space="PSUM") as prep_ps,
    ):
        for ke in range(NCH):
            row = prep_sb.tile([128, DIM], F32, tag="w3row")
            nc.sync.dma_start(out=row[:], in_=w3_d[ke * 128 : (ke + 1) * 128, :])
            for kd in range(NCH):
                ps = prep_ps.tile([128, 128], F32, tag="tr")
                nc.tensor.transpose(ps[:], row[:, kd * 128 : (kd + 1) * 128], ident[:])
                nc.scalar.copy(w3t[kd][:, ke * 128 : (ke + 1) * 128], ps[:])

        for kd in range(NCH):
            row = prep_sb.tile([128, WDIM], F32, tag="w1row")
            nc.sync.dma_start(out=row[:], in_=w1_d[kd * 128 : (kd + 1) * 128, :])
            for wc, (woff, wsz) in enumerate(WCH):
                ps = prep_ps.tile([128, 128], F32, tag="tr")
                nc.tensor.transpose(ps[:wsz, :], row[:, woff : woff + wsz], ident[:])
                nc.scalar.copy(w1t[wc][:wsz, kd * 128 : (kd + 1) * 128], ps[:wsz, :])

        for b in range(BPC):
            for sc, (soff, ssz) in enumerate(SCH):
                ts_ = prep_sb.tile([128, DIM], F32, tag="tokrow")
                nc.sync.dma_start(out=ts_[:ssz, :], in_=token_d[b, soff : soff + ssz, :])
                for kd in range(NCH):
                    ps = prep_ps.tile([128, 128], F32, tag="tr")
                    nc.tensor.transpose(
                        ps[:, :ssz],
                        ts_[:ssz, kd * 128 : (kd + 1) * 128],
                        ident[:ssz, :ssz],
                    )
                    nc.scalar.copy(tokT[b][kd][:, soff : soff + ssz], ps[:, :ssz])
            for kd in range(NCH):
                nc.vector.tensor_copy(tokB[b][kd][:], tokT[b][kd][:])

            lgrow = prep_sb.tile([128, WDIM], F32, tag="lgrow")
            nc.sync.dma_start(out=lgrow[:CLS, :], in_=lg_d[b, :, :])
            lgt = []
            for wc, (woff, wsz) in enumerate(WCH):
                ps = prep_ps.tile([128, 128], F32, tag="tr")
                nc.tensor.transpose(
                    ps[:wsz, :CLS], lgrow[:CLS, woff : woff + wsz], ident[:CLS, :CLS]
                )
                t = prep_sb.tile([128, CLS], F32, tag=f"lgt{wc}")
                nc.scalar.copy(t[:wsz, :], ps[:wsz, :CLS])
                lgt.append(t)
            for kd in range(NCH):
                ps = prep_ps.tile([128, CLS], F32, tag="lab")
                for wc, (woff, wsz) in enumerate(WCH):
                    nc.tensor.matmul(
                        ps[:],
                        w1t[wc][:wsz, kd * 128 : (kd + 1) * 128],
                        lgt[wc][:wsz, :],
                        start=(wc == 0),
                        stop=(wc == len(WCH) - 1),
                    )
                nc.scalar.copy(labT[b][kd][:], ps[:])

    # ---------------- main ----------------
    NPAIR = CLS // 2  # 40 pairs per batch
    OCT = 4  # pairs per octet group (8 classes)
    with tc.tile_pool(name="accps", bufs=1, space="PSUM") as acc_pool, tc.tile_pool(
        name="fin_sb", bufs=2
    ) as fin_sb:
        for b in range(BPC):
            acc_ps = acc_pool.tile([128, NCH, 512], F32, tag="acc", name=f"accps{b}")
            with (
                tc.tile_pool(name="stage", bufs=2) as stage_pool,
                tc.tile_pool(name="prod", bufs=2) as prod_pool,
                tc.tile_pool(name="epool", bufs=3) as e_pool,
                tc.tile_pool(name="zpool", bufs=4) as z_pool,
                tc.tile_pool(name="feat", bufs=2, space="PSUM") as feat_pool,
            ):
                pending = None  # (p, E) whose identity-accumulate is deferred

                def flush_pending():
                    nonlocal pending
                    if pending is None:
                        return
                    pp, pE = pending
                    for m in range(NCH):
                        nc.tensor.matmul(
                            acc_ps[:, m, 0 : 2 * IMG],
                            ident_bf[:],
                            pE[:, m, :],
                            start=(pp == 0),
                            stop=(pp == NPAIR - 1),
                            skip_group_check=True,
                        )
                    pending = None

                for g in range(NPAIR // OCT):  # octet groups
                    stage = stage_pool.tile([128, NCH, OCT * 2 * IMG], BF16, tag="stage")
                    prod = prod_pool.tile([128, NCH, OCT * 2 * IMG], BF16, tag="prod")
                    for m in range(NCH):
                        for j in range(2 * OCT):
                            c = g * 2 * OCT + j
                            nc.vector.tensor_scalar_mul(
                                stage[:, m, j * IMG : (j + 1) * IMG],
                                tokB[b][m][:],
                                labT[b][m][:, c : c + 1],
                            )
                        nc.scalar.activation(prod[:, m, :], stage[:, m, :], TS.Tanh)
                    for pj in range(OCT):
                        p = g * OCT + pj
                        kf = int(round(zdve_frac * NPAIR))
                        z_dve = ((p + 1) * kf) // NPAIR > (p * kf) // NPAIR
                        E = e_pool.tile([128, NCH, 2 * IMG], BF16, tag="E", name=f"E{b}_{p}")
                        Z = z_pool.tile([128, 8], F32, tag="Z", name=f"Z{b}_{p}")
                        R = z_pool.tile([128, 8], F32, tag="R", name=f"R{b}_{p}")
                        for half in range(2):
                            feat = feat_pool.tile(
                                [128, 2, 512], F32, tag="feat", name=f"feat{b}_{p}_{half}"
                            )
                            for mi in range(2):
                                m = half * 2 + mi
                                for k in range(NCH):
                                    nc.tensor.matmul(
                                        feat[:, mi, 0 : 2 * IMG],
                                        w3t[k][:, m * 128 : (m + 1) * 128],
                                        prod[:, k, pj * 2 * IMG : (pj + 1) * 2 * IMG],
                                        start=(k == 0),
                                        stop=(k == NCH - 1),
                                    )
                            if half == 1:
                                # previous pair's deferred accumulate rides here,
                                # after this pair's main matmuls are queued
                                flush_pending()
                            if z_dve:
                                nc.scalar.activation(
                                    E[:, half * 2 : half * 2 + 2, :],
                                    feat[:, :, 0 : 2 * IMG],
                                    TS.Exp,
                                )
                            else:
                                for mi in range(2):
                                    m = half * 2 + mi
                                    for j in range(2):
                                        nc.scalar.activation(
                                            E[:, m, j * IMG : (j + 1) * IMG],
                                            feat[:, mi, j * IMG : (j + 1) * IMG],
                                            TS.Exp,
                                            accum_out=Z[:, 2 * m + j : 2 * m + j + 1],
                                        )
                        if z_dve:
                            nc.vector.tensor_reduce(
                                Z[:],
                                E[:].rearrange("p m (j s) -> p m j s", j=2),
                                axis=X,
                                op=AO.add,
                            )
                        nc.vector.reciprocal(R[:], Z[:])
                        # scale E in place; identity-accumulate is deferred one pair
                        for m in range(NCH):
                            for j in range(2):
                                nc.vector.tensor_scalar_mul(
                                    E[:, m, j * IMG : (j + 1) * IMG],
                                    E[:, m, j * IMG : (j + 1) * IMG],
                                    R[:, 2 * m + j : 2 * m + j + 1],
                                )
                        pending = (p, E)
                flush_pending()

            # ---------------- final for this batch ----------------
            with tc.tile_pool(name="fin_ps", bufs=4, space="PSUM") as fin_ps:
                outT = fin_sb.tile([128, NCH, IMG], F32, tag="outT", name=f"outT{b}")
                tmp2 = fin_sb.tile([128, NCH, 2 * IMG], F32, tag="tmpacc", name=f"tmpacc{b}")
                tmp = fin_sb.tile([128, NCH, IMG], F32, tag="tmpsum", name=f"tmpsum{b}")
                for m in range(NCH):
                    nc.scalar.copy(tmp2[:, m, :], acc_ps[:, m, 0 : 2 * IMG])
                    nc.vector.tensor_tensor(
                        tmp[:, m, :], tmp2[:, m, 0:IMG], tmp2[:, m, IMG : 2 * IMG], AO.add
                    )
                    nc.vector.tensor_tensor(
                        outT[:, m, :], tokT[b][m][:], tmp[:, m, :], AO.mult
                    )
                for sc, (soff, ssz) in enumerate(SCH):
                    outs = fin_sb.tile([128, DIM], F32, tag=f"outs{sc}", name=f"outs{b}_{sc}")
                    for m in range(NCH):
                        ps = fin_ps.tile([128, 128], F32, tag="tr")
                        nc.tensor.transpose(
                            ps[:ssz, :], outT[:, m, soff : soff + ssz], ident[:]
                        )
                        nc.scalar.copy(outs[:ssz, m * 128 : (m + 1) * 128], ps[:ssz, :])
                    nc.sync.dma_start(out=out_d[b, soff : soff + ssz, :], in_=outs[:ssz, :])


_NC_CACHE = {}


def _get_nc(mm_dt=MM_DT, variant="v2", zdve_frac=0.64):
    key = (str(mm_dt), variant, zdve_frac)
    if key not in _NC_CACHE:
        _NC_CACHE[key] = build_kernel(mm_dt, variant, zdve_frac)
    return _NC_CACHE[key]


def run(inputs, trace=False, mm_dt=MM_DT, variant="v2", zdve_frac=0.64):
    token = np.ascontiguousarray(np.asarray(inputs["tokenFeaturemap"], np.float32))
    lg = np.ascontiguousarray(np.asarray(inputs["labelGraphfeatures"], np.float32))
    w1 = np.ascontiguousarray(np.asarray(inputs["W1"], np.float32))
    w3 = np.ascontiguousarray(np.asarray(inputs["W3"], np.float32))
    nc = _get_nc(mm_dt, variant, zdve_frac)
    in_maps = [
        {
            "token": token[i * BPC : (i + 1) * BPC],
            "lg": lg[i * BPC : (i + 1) * BPC],
            "W1": w1,
            "W3": w3,
        }
        for i in range(N_CORES)
    ]
    res = run_bass_kernel_spmd(nc, in_maps, list(range(N_CORES)), trace=trace)
    out = np.concatenate([res.results[i]["out"] for i in range(N_CORES)], axis=0)
    return out, res


def kernel(**inputs) -> np.ndarray:
    out, _ = run(inputs)
    return out


# revision 14
# speedup vs baseline: 2.0165x; 1.0267x over previous
import os
import sys

for _p in ("/opt/trn_rl_repo", "/root/.axon_site/_ro/trn_rl_repo"):
    if os.path.isdir(_p) and _p not in sys.path:
        sys.path.append(_p)

import numpy as np
import concourse.bass as bass
import concourse.mybir as mybir
import concourse.tile as tile
from concourse import masks
from concourse.bass_utils import run_bass_kernel_spmd

# Problem shapes (hardcoded per contract)
BS, IMG, CLS, DIM, WDIM = 16, 196, 80, 512, 300
N_CORES = 8
BPC = BS // N_CORES  # batches per core
NCH = DIM // 128  # 4 d-chunks of 128
SCH = [(0, 128), (128, 68)]  # s-chunks (offset, size) for IMG=196
WCH = [(0, 128), (128, 128), (256, 44)]  # w-chunks for WDIM=300

F32 = mybir.dt.float32
# dtype used for the fc3 matmul operands (W3T weights and prodT moving data)
MM_DT = mybir.dt.float32


def split_multi_waits(nc):
    """This walrus build accepts a single sync-wait per instruction on the
    CTRL encodings; split extra waits into single-wait NoOps on the same
    engine immediately before the instruction."""
    k = 0
    for f in nc.m.functions:
        for bb in f.blocks:
            il = bb.instructions
            i = 0
            while i < len(il):
                ins = il[i]
                si = ins.sync_info
                if si is not None and len(si.on_wait) > 1:
                    waits = list(si.on_wait)
                    for w in waits[:-1]:
                        nop = mybir.InstNoOp(name=f"waitsplit_{k}", ins=[], outs=[])
                        k += 1
                        nop.engine = ins.engine
                        nop.sync_info = mybir.SyncInfo(on_wait=[w], on_update=[])
                        il.insert(i, nop)
                        i += 1
                    ins.sync_info = mybir.SyncInfo(
                        on_wait=[waits[-1]], on_update=list(si.on_update)
                    )
                i += 1
    return k


def build_kernel(mm_dt=MM_DT, variant="v2", zdve_frac=0.64):
    nc = bass.Bass("TRN2", target_bir_lowering=False, debug=False, num_devices=N_CORES)
    token_d = nc.dram_tensor("token", [BPC, IMG, DIM], F32, kind="ExternalInput").ap()
    lg_d = nc.dram_tensor("lg", [BPC, CLS, WDIM], F32, kind="ExternalInput").ap()
    w1_d = nc.dram_tensor("W1", [DIM, WDIM], F32, kind="ExternalInput").ap()
    w3_d = nc.dram_tensor("W3", [DIM, DIM], F32, kind="ExternalInput").ap()
    out_d = nc.dram_tensor("out", [BPC, IMG, DIM], F32, kind="ExternalOutput").ap()

    with tile.TileContext(nc) as tc:
        if variant == "v1":
            emit_body(tc, nc, token_d, lg_d, w1_d, w3_d, out_d, mm_dt)
        else:
            emit_body_v2(tc, nc, token_d, lg_d, w1_d, w3_d, out_d, zdve_frac)
    split_multi_waits(nc)
    return nc


def emit_body(tc, nc, token_d, lg_d, w1_d, w3_d, out_d, mm_dt):
    TS = mybir.ActivationFunctionType
    AO = mybir.AluOpType

    with tc.tile_pool(name="const", bufs=1) as const_pool:
        _emit_body_inner(tc, nc, token_d, lg_d, w1_d, w3_d, out_d, mm_dt, const_pool)


def _emit_body_inner(tc, nc, token_d, lg_d, w1_d, w3_d, out_d, mm_dt, const_pool):
    TS = mybir.ActivationFunctionType
    AO = mybir.AluOpType

    ident = const_pool.tile([128, 128], F32)
    masks.make_identity(nc, ident[:])

    # Persistent SBUF tensors
    w3t = [const_pool.tile([128, DIM], mm_dt, tag=f"w3t{k}", name=f"w3t{k}") for k in range(NCH)]
    w1t = [const_pool.tile([128, DIM], F32, tag=f"w1t{w}", name=f"w1t{w}") for w in range(len(WCH))]
    tokT = [
        [const_pool.tile([128, IMG], F32, tag=f"tokT{b}_{k}", name=f"tokT{b}_{k}") for k in range(NCH)]
        for b in range(BPC)
    ]
    labT = [
        [const_pool.tile([128, CLS], F32, tag=f"labT{b}_{k}", name=f"labT{b}_{k}") for k in range(NCH)]
        for b in range(BPC)
    ]
    acc = [const_pool.tile([128, NCH, IMG], F32, tag=f"acc{b}", name=f"acc{b}") for b in range(BPC)]

    # ---------------- prep phase ----------------
    with (
        tc.tile_pool(name="prep_sb", bufs=4) as prep_sb,
        tc.tile_pool(name="prep_ps", bufs=4, space="PSUM") as prep_ps,
    ):
        # W3T[kd][:, ke*128:+128] = W3[ke-chunk, kd-chunk].T
        for ke in range(NCH):
            row = prep_sb.tile([128, DIM], F32, tag="w3row")
            nc.sync.dma_start(out=row[:], in_=w3_d[ke * 128 : (ke + 1) * 128, :])
            for kd in range(NCH):
                ps = prep_ps.tile([128, 128], F32, tag="tr")
                nc.tensor.transpose(
                    ps[:], row[:, kd * 128 : (kd + 1) * 128], ident[:]
                )
                nc.scalar.copy(w3t[kd][:, ke * 128 : (ke + 1) * 128], ps[:])

        # W1T[wc][0:wsz, kd*128:+128] = W1[kd-chunk, w-chunk].T
        for kd in range(NCH):
            row = prep_sb.tile([128, WDIM], F32, tag="w1row")
            nc.sync.dma_start(out=row[:], in_=w1_d[kd * 128 : (kd + 1) * 128, :])
            for wc, (woff, wsz) in enumerate(WCH):
                ps = prep_ps.tile([128, 128], F32, tag="tr")
                nc.tensor.transpose(
                    ps[:wsz, :], row[:, woff : woff + wsz], ident[:]
                )
                nc.scalar.copy(w1t[wc][:wsz, kd * 128 : (kd + 1) * 128], ps[:wsz, :])

        for b in range(BPC):
            # tokenT
            for sc, (soff, ssz) in enumerate(SCH):
                ts = prep_sb.tile([128, DIM], F32, tag="tokrow")
                nc.sync.dma_start(
                    out=ts[:ssz, :], in_=token_d[b, soff : soff + ssz, :]
                )
                for kd in range(NCH):
                    ps = prep_ps.tile([128, 128], F32, tag="tr")
                    nc.tensor.transpose(
                        ps[:, :ssz],
                        ts[:ssz, kd * 128 : (kd + 1) * 128],
                        ident[:ssz, :ssz],
                    )
                    nc.scalar.copy(tokT[b][kd][:, soff : soff + ssz], ps[:, :ssz])

            # lgT then labT = (lg @ W1.T).T
            lgrow = prep_sb.tile([128, WDIM], F32, tag="lgrow")
            nc.sync.dma_start(out=lgrow[:CLS, :], in_=lg_d[b, :, :])
            lgt = []
            for wc, (woff, wsz) in enumerate(WCH):
                ps = prep_ps.tile([128, 128], F32, tag="tr")
                nc.tensor.transpose(
                    ps[:wsz, :CLS],
                    lgrow[:CLS, woff : woff + wsz],
                    ident[:CLS, :CLS],
                )
                t = prep_sb.tile([128, CLS], F32, tag=f"lgt{wc}")
                nc.scalar.copy(t[:wsz, :], ps[:wsz, :CLS])
                lgt.append(t)
            for kd in range(NCH):
                ps = prep_ps.tile([128, CLS], F32, tag="lab")
                for wc, (woff, wsz) in enumerate(WCH):
                    nc.tensor.matmul(
                        ps[:],
                        w1t[wc][:wsz, kd * 128 : (kd + 1) * 128],
                        lgt[wc][:wsz, :],
                        start=(wc == 0),
                        stop=(wc == len(WCH) - 1),
                    )
                nc.scalar.copy(labT[b][kd][:], ps[:])

            nc.vector.memset(acc[b][:], 0.0)

    # ---------------- main loop ----------------
    with (
        tc.tile_pool(name="stage", bufs=2) as stage_pool,
        tc.tile_pool(name="prod", bufs=2) as prod_pool,
        tc.tile_pool(name="epool", bufs=2) as e_pool,
        tc.tile_pool(name="zpool", bufs=3) as z_pool,
        tc.tile_pool(name="feat", bufs=2, space="PSUM") as feat_pool,
    ):
        for b in range(BPC):
            for p in range(CLS // 2):
                cA, cB = 2 * p, 2 * p + 1
                stage = stage_pool.tile([128, NCH, 2 * IMG], F32, tag="stage")
                for m in range(NCH):
                    nc.vector.tensor_scalar_mul(
                        stage[:, m, 0:IMG], tokT[b][m][:], labT[b][m][:, cA : cA + 1]
                    )
                    nc.vector.tensor_scalar_mul(
                        stage[:, m, IMG : 2 * IMG],
                        tokT[b][m][:],
                        labT[b][m][:, cB : cB + 1],
                    )
                prod = prod_pool.tile([128, NCH, 2 * IMG], mm_dt, tag="prod")
                for m in range(NCH):
                    nc.scalar.activation(prod[:, m, :], stage[:, m, :], TS.Tanh)

                feat = feat_pool.tile([128, NCH, 512], F32, tag="feat")
                for m in range(NCH):
                    for k in range(NCH):
                        nc.tensor.matmul(
                            feat[:, m, 0 : 2 * IMG],
                            w3t[k][:, m * 128 : (m + 1) * 128],
                            prod[:, k, :],
                            start=(k == 0),
                            stop=(k == NCH - 1),
                        )

                E = e_pool.tile([128, NCH, 2 * IMG], F32, tag="E")
                Z = z_pool.tile([128, 8], F32, tag="Z")
                for m in range(NCH):
                    nc.scalar.activation(
                        E[:, m, 0:IMG],
                        feat[:, m, 0:IMG],
                        TS.Exp,
                        accum_out=Z[:, 2 * m : 2 * m + 1],
                    )
                    nc.scalar.activation(
                        E[:, m, IMG : 2 * IMG],
                        feat[:, m, IMG : 2 * IMG],
                        TS.Exp,
                        accum_out=Z[:, 2 * m + 1 : 2 * m + 2],
                    )
                R = z_pool.tile([128, 8], F32, tag="R")
                nc.vector.reciprocal(R[:], Z[:])
                for m in range(NCH):
                    nc.vector.scalar_tensor_tensor(
                        acc[b][:, m, :],
                        E[:, m, 0:IMG],
                        R[:, 2 * m : 2 * m + 1],
                        acc[b][:, m, :],
                        AO.mult,
                        AO.add,
                    )
                    nc.vector.scalar_tensor_tensor(
                        acc[b][:, m, :],
                        E[:, m, IMG : 2 * IMG],
                        R[:, 2 * m + 1 : 2 * m + 2],
                        acc[b][:, m, :],
                        AO.mult,
                        AO.add,
                    )

    # ---------------- final phase ----------------
    with (
        tc.tile_pool(name="fin_sb", bufs=2) as fin_sb,
        tc.tile_pool(name="fin_ps", bufs=4, space="PSUM") as fin_ps,
    ):
        for b in range(BPC):
            outT = fin_sb.tile([128, NCH, IMG], F32, tag="outT")
            for m in range(NCH):
                nc.vector.tensor_tensor(
                    outT[:, m, :],
                    tokT[b][m][:],
                    acc[b][:, m, :],
                    mybir.AluOpType.mult,
                )
            for sc, (soff, ssz) in enumerate(SCH):
                outs = fin_sb.tile([128, DIM], F32, tag=f"outs{sc}")
                for m in range(NCH):
                    ps = fin_ps.tile([128, 128], F32, tag="tr")
                    nc.tensor.transpose(
                        ps[:ssz, :], outT[:, m, soff : soff + ssz], ident[:]
                    )
                    nc.scalar.copy(outs[:ssz, m * 128 : (m + 1) * 128], ps[:ssz, :])
                nc.sync.dma_start(
                    out=out_d[b, soff : soff + ssz, :], in_=outs[:ssz, :]
                )


def emit_body_v2(tc, nc, token_d, lg_d, w1_d, w3_d, out_d, zdve_frac=0.64):
    with tc.tile_pool(name="const", bufs=1) as const_pool:
        _emit_v2_inner(tc, nc, token_d, lg_d, w1_d, w3_d, out_d, const_pool, zdve_frac)


def _emit_v2_inner(tc, nc, token_d, lg_d, w1_d, w3_d, out_d, cp, zdve_frac):
    TS = mybir.ActivationFunctionType
    AO = mybir.AluOpType
    BF16 = mybir.dt.bfloat16
    X = mybir.AxisListType.X

    ident = cp.tile([128, 128], F32)
    masks.make_identity(nc, ident[:])
    ident_bf = cp.tile([128, 128], BF16)
    masks.make_identity(nc, ident_bf[:])

    w3t = [cp.tile([128, DIM], BF16, tag=f"w3t{k}", name=f"w3t{k}") for k in range(NCH)]
    w1t = [cp.tile([128, DIM], F32, tag=f"w1t{w}", name=f"w1t{w}") for w in range(len(WCH))]
    tokT = [
        [cp.tile([128, IMG], F32, tag=f"tokT{b}_{k}", name=f"tokT{b}_{k}") for k in range(NCH)]
        for b in range(BPC)
    ]
    tokB = [
        [cp.tile([128, IMG], BF16, tag=f"tokB{b}_{k}", name=f"tokB{b}_{k}") for k in range(NCH)]
        for b in range(BPC)
    ]
    labT = [
        [cp.tile([128, CLS], F32, tag=f"labT{b}_{k}", name=f"labT{b}_{k}") for k in range(NCH)]
        for b in range(BPC)
    ]

    # ---------------- prep ----------------
    with (
        tc.tile_pool(name="prep_sb", bufs=4) as prep_sb,
        tc.tile_pool(name="prep_ps", bufs=4, space="PSUM") as prep_ps,
    ):
        _cp_i = [0]

        def pcopy(dst, src_):
            eng = nc.scalar if _cp_i[0] % 2 == 0 else nc.vector
            _cp_i[0] += 1
            if eng is nc.scalar:
                eng.copy(dst, src_)
            else:
                eng.tensor_copy(dst, src_)

        for ke in range(NCH):
            row = prep_sb.tile([128, DIM], F32, tag="w3row")
            nc.sync.dma_start(out=row[:], in_=w3_d[ke * 128 : (ke + 1) * 128, :])
            for kd in range(NCH):
                ps = prep_ps.tile([128, 128], F32, tag="tr")
                nc.tensor.transpose(ps[:], row[:, kd * 128 : (kd + 1) * 128], ident[:])
                pcopy(w3t[kd][:, ke * 128 : (ke + 1) * 128], ps[:])

        for kd in range(NCH):
            row = prep_sb.tile([128, WDIM], F32, tag="w1row")
            nc.sync.dma_start(out=row[:], in_=w1_d[kd * 128 : (kd + 1) * 128, :])
            for wc, (woff, wsz) in enumerate(WCH):
                ps = prep_ps.tile([128, 128], F32, tag="tr")
                nc.tensor.transpose(ps[:wsz, :], row[:, woff : woff + wsz], ident[:])
                pcopy(w1t[wc][:wsz, kd * 128 : (kd + 1) * 128], ps[:wsz, :])

        for b in range(BPC):
            for sc, (soff, ssz) in enumerate(SCH):
                ts_ = prep_sb.tile([128, DIM], F32, tag="tokrow")
                nc.sync.dma_start(out=ts_[:ssz, :], in_=token_d[b, soff : soff + ssz, :])
                for kd in range(NCH):
                    ps = prep_ps.tile([128, 128], F32, tag="tr")
                    nc.tensor.transpose(
                        ps[:, :ssz],
                        ts_[:ssz, kd * 128 : (kd + 1) * 128],
                        ident[:ssz, :ssz],
                    )
                    pcopy(tokT[b][kd][:, soff : soff + ssz], ps[:, :ssz])
            for kd in range(NCH):
                nc.vector.tensor_copy(tokB[b][kd][:], tokT[b][kd][:])

            lgrow = prep_sb.tile([128, WDIM], F32, tag="lgrow")
            nc.sync.dma_start(out=lgrow[:CLS, :], in_=lg_d[b, :, :])
            lgt = []
            for wc, (woff, wsz) in enumerate(WCH):
                ps = prep_ps.tile([128, 128], F32, tag="tr")
                nc.tensor.transpose(
                    ps[:wsz, :CLS], lgrow[:CLS, woff : woff + wsz], ident[:CLS, :CLS]
                )
                t = prep_sb.tile([128, CLS], F32, tag=f"lgt{wc}")
                pcopy(t[:wsz, :], ps[:wsz, :CLS])
                lgt.append(t)
            for kd in range(NCH):
                ps = prep_ps.tile([128, CLS], F32, tag="lab")
                for wc, (woff, wsz) in enumerate(WCH):
                    nc.tensor.matmul(
                        ps[:],
                        w1t[wc][:wsz, kd * 128 : (kd + 1) * 128],
                        lgt[wc][:wsz, :],
                        start=(wc == 0),
                        stop=(wc == len(WCH) - 1),
                    )
                pcopy(labT[b][kd][:], ps[:])

    # ---------------- main ----------------
    NPAIR = CLS // 2  # 40 pairs per batch
    OCT = 4  # pairs per octet group (8 classes)
    with tc.tile_pool(name="accps", bufs=1, space="PSUM") as acc_pool, tc.tile_pool(
        name="fin_sb", bufs=2
    ) as fin_sb:
        # acc slot for chunk m: bank m//2, free offset (m%2)*256, length IMG
        def acc_slot(acc_ps, m):
            return acc_ps[:, m // 2, (m % 2) * 256 : (m % 2) * 256 + IMG]

        for b in range(BPC):
            acc_ps = acc_pool.tile([128, 2, 512], F32, tag="acc", name=f"accps{b}")
            with (
                tc.tile_pool(name="stage", bufs=2) as stage_pool,
                tc.tile_pool(name="prod", bufs=2) as prod_pool,
                tc.tile_pool(name="epool", bufs=3) as e_pool,
                tc.tile_pool(name="zpool", bufs=4) as z_pool,
                tc.tile_pool(name="feat", bufs=3, space="PSUM") as feat_pool,
            ):
                pending = None  # (p, E) whose identity-accumulate is deferred

                def flush_pending():
                    nonlocal pending
                    if pending is None:
                        return
                    pp, pE = pending
                    for m in range(NCH):
                        for j in range(2):
                            nc.tensor.matmul(
                                acc_slot(acc_ps, m),
                                ident_bf[:],
                                pE[:, m, j * IMG : (j + 1) * IMG],
                                start=(pp == 0 and j == 0),
                                stop=(pp == NPAIR - 1 and j == 1),
                                skip_group_check=True,
                            )
                    pending = None

                for g in range(NPAIR // OCT):  # octet groups
                    stage = stage_pool.tile([128, NCH, OCT * 2 * IMG], BF16, tag="stage")
                    prod = prod_pool.tile([128, NCH, OCT * 2 * IMG], BF16, tag="prod")
                    for m in range(NCH):
                        for j in range(2 * OCT):
                            c = g * 2 * OCT + j
                            nc.vector.tensor_scalar_mul(
                                stage[:, m, j * IMG : (j + 1) * IMG],
                                tokB[b][m][:],
                                labT[b][m][:, c : c + 1],
                            )
                        nc.scalar.activation(prod[:, m, :], stage[:, m, :], TS.Tanh)
                    for pj in range(OCT):
                        p = g * OCT + pj
                        kf = int(round(zdve_frac * NPAIR))
                        z_dve = ((p + 1) * kf) // NPAIR > (p * kf) // NPAIR
                        E = e_pool.tile([128, NCH, 2 * IMG], BF16, tag="E", name=f"E{b}_{p}")
                        Z = z_pool.tile([128, 8], F32, tag="Z", name=f"Z{b}_{p}")
                        R = z_pool.tile([128, 8], F32, tag="R", name=f"R{b}_{p}")
                        for half in range(2):
                            feat = feat_pool.tile(
                                [128, 2, 512], F32, tag="feat", name=f"feat{b}_{p}_{half}"
                            )
                            for mi in range(2):
                                m = half * 2 + mi
                                for k in range(NCH):
                                    nc.tensor.matmul(
                                        feat[:, mi, 0 : 2 * IMG],
                                        w3t[k][:, m * 128 : (m + 1) * 128],
                                        prod[:, k, pj * 2 * IMG : (pj + 1) * 2 * IMG],
                                        start=(k == 0),
                                        stop=(k == NCH - 1),
                                    )
                            if half == 1:
                                # previous pair's deferred accumulate rides here,
                                # after this pair's main matmuls are queued
                                flush_pending()
                            if z_dve:
                                nc.scalar.activation(
                                    E[:, half * 2 : half * 2 + 2, :],
                                    feat[:, :, 0 : 2 * IMG],
                                    TS.Exp,
                                )
                            else:
                                for mi in range(2):
                                    m = half * 2 + mi
                                    for j in range(2):
                                        nc.scalar.activation(
                                            E[:, m, j * IMG : (j + 1) * IMG],
                                            feat[:, mi, j * IMG : (j + 1) * IMG],
                                            TS.Exp,
                                            accum_out=Z[:, 2 * m + j : 2 * m + j + 1],
                                        )
                        if z_dve:
                            nc.vector.tensor_reduce(
                                Z[:],
                                E[:].rearrange("p m (j s) -> p m j s", j=2),
                                axis=X,
                                op=AO.add,
                            )
                        nc.vector.reciprocal(R[:], Z[:])
                        # scale E in place; identity-accumulate is deferred one pair
                        for m in range(NCH):
                            for j in range(2):
                                nc.vector.tensor_scalar_mul(
                                    E[:, m, j * IMG : (j + 1) * IMG],
                                    E[:, m, j * IMG : (j + 1) * IMG],
                                    R[:, 2 * m + j : 2 * m + j + 1],
                                )
                        pending = (p, E)
                flush_pending()

            # ---------------- final for this batch ----------------
            with tc.tile_pool(name="fin_ps", bufs=4, space="PSUM") as fin_ps:
                outT = fin_sb.tile([128, NCH, IMG], F32, tag="outT", name=f"outT{b}")
                for m in range(NCH):
                    nc.vector.tensor_tensor(
                        outT[:, m, :], tokT[b][m][:], acc_slot(acc_ps, m), AO.mult
                    )
                for sc, (soff, ssz) in enumerate(SCH):
                    outs = fin_sb.tile([128, DIM], F32, tag=f"outs{sc}", name=f"outs{b}_{sc}")
                    for m in range(NCH):
                        ps = fin_ps.tile([128, 128], F32, tag="tr")
                        nc.tensor.transpose(
                            ps[:ssz, :], outT[:, m, soff : soff + ssz], ident[:]
                        )
                        nc.scalar.copy(outs[:ssz, m * 128 : (m + 1) * 128], ps[:ssz, :])
                    nc.sync.dma_start(out=out_d[b, soff : soff + ssz, :], in_=outs[:ssz, :])


_NC_CACHE = {}


def _get_nc(mm_dt=MM_DT, variant="v2", zdve_frac=0.64):
    key = (str(mm_dt), variant, zdve_frac)
    if key not in _NC_CACHE:
        _NC_CACHE[key] = build_kernel(mm_dt, variant, zdve_frac)
    return _NC_CACHE[key]


def run(inputs, trace=False, mm_dt=MM_DT, variant="v2", zdve_frac=0.64):
    token = np.ascontiguousarray(np.asarray(inputs["tokenFeaturemap"], np.float32))
    lg = np.ascontiguousarray(np.asarray(inputs["labelGraphfeatures"], np.float32))
    w1 = np.ascontiguousarray(np.asarray(inputs["W1"], np.float32))
    w3 = np.ascontiguousarray(np.asarray(inputs["W3"], np.float32))
    nc = _get_nc(mm_dt, variant, zdve_frac)
    in_maps = [
        {
            "token": token[i * BPC : (i + 1) * BPC],
            "lg": lg[i * BPC : (i + 1) * BPC],
            "W1": w1,
            "W3": w3,
        }
        for i in range(N_CORES)
    ]
    res = run_bass_kernel_spmd(nc, in_maps, list(range(N_CORES)), trace=trace)
    out = np.concatenate([res.results[i]["out"] for i in range(N_CORES)], axis=0)
    return out, res


def kernel(**inputs) -> np.ndarray:
    out, _ = run(inputs)
    return out


# revision 15
# speedup vs baseline: 2.0227x; 1.0031x over previous
import os
import sys

for _p in ("/opt/trn_rl_repo", "/root/.axon_site/_ro/trn_rl_repo"):
    if os.path.isdir(_p) and _p not in sys.path:
        sys.path.append(_p)

import numpy as np
import concourse.bass as bass
import concourse.mybir as mybir
import concourse.tile as tile
from concourse import masks
from concourse.bass_utils import run_bass_kernel_spmd

# Problem shapes (hardcoded per contract)
BS, IMG, CLS, DIM, WDIM = 16, 196, 80, 512, 300
N_CORES = 8
BPC = BS // N_CORES  # batches per core
NCH = DIM // 128  # 4 d-chunks of 128
SCH = [(0, 128), (128, 68)]  # s-chunks (offset, size) for IMG=196
WCH = [(0, 128), (128, 128), (256, 44)]  # w-chunks for WDIM=300

F32 = mybir.dt.float32
# dtype used for the fc3 matmul operands (W3T weights and prodT moving data)
MM_DT = mybir.dt.float32


def split_multi_waits(nc):
    """This walrus build accepts a single sync-wait per instruction on the
    CTRL encodings; split extra waits into single-wait NoOps on the same
    engine immediately before the instruction."""
    k = 0
    for f in nc.m.functions:
        for bb in f.blocks:
            il = bb.instructions
            i = 0
            while i < len(il):
                ins = il[i]
                si = ins.sync_info
                if si is not None and len(si.on_wait) > 1:
                    waits = list(si.on_wait)
                    for w in waits[:-1]:
                        nop = mybir.InstNoOp(name=f"waitsplit_{k}", ins=[], outs=[])
                        k += 1
                        nop.engine = ins.engine
                        nop.sync_info = mybir.SyncInfo(on_wait=[w], on_update=[])
                        il.insert(i, nop)
                        i += 1
                    ins.sync_info = mybir.SyncInfo(
                        on_wait=[waits[-1]], on_update=list(si.on_update)
                    )
                i += 1
    return k


def build_kernel(mm_dt=MM_DT, variant="v2", zdve_frac=0.64):
    nc = bass.Bass("TRN2", target_bir_lowering=False, debug=False, num_devices=N_CORES)
    token_d = nc.dram_tensor("token", [BPC, IMG, DIM], F32, kind="ExternalInput").ap()
    lg_d = nc.dram_tensor("lg", [BPC, CLS, WDIM], F32, kind="ExternalInput").ap()
    w1_d = nc.dram_tensor("W1", [DIM, WDIM], F32, kind="ExternalInput").ap()
    w3_d = nc.dram_tensor("W3", [DIM, DIM], F32, kind="ExternalInput").ap()
    out_d = nc.dram_tensor("out", [BPC, IMG, DIM], F32, kind="ExternalOutput").ap()

    with tile.TileContext(nc) as tc:
        if variant == "v1":
            emit_body(tc, nc, token_d, lg_d, w1_d, w3_d, out_d, mm_dt)
        else:
            emit_body_v2(tc, nc, token_d, lg_d, w1_d, w3_d, out_d, zdve_frac)
    split_multi_waits(nc)
    return nc


def emit_body(tc, nc, token_d, lg_d, w1_d, w3_d, out_d, mm_dt):
    TS = mybir.ActivationFunctionType
    AO = mybir.AluOpType

    with tc.tile_pool(name="const", bufs=1) as const_pool:
        _emit_body_inner(tc, nc, token_d, lg_d, w1_d, w3_d, out_d, mm_dt, const_pool)


def _emit_body_inner(tc, nc, token_d, lg_d, w1_d, w3_d, out_d, mm_dt, const_pool):
    TS = mybir.ActivationFunctionType
    AO = mybir.AluOpType

    ident = const_pool.tile([128, 128], F32)
    masks.make_identity(nc, ident[:])

    # Persistent SBUF tensors
    w3t = [const_pool.tile([128, DIM], mm_dt, tag=f"w3t{k}", name=f"w3t{k}") for k in range(NCH)]
    w1t = [const_pool.tile([128, DIM], F32, tag=f"w1t{w}", name=f"w1t{w}") for w in range(len(WCH))]
    tokT = [
        [const_pool.tile([128, IMG], F32, tag=f"tokT{b}_{k}", name=f"tokT{b}_{k}") for k in range(NCH)]
        for b in range(BPC)
    ]
    labT = [
        [const_pool.tile([128, CLS], F32, tag=f"labT{b}_{k}", name=f"labT{b}_{k}") for k in range(NCH)]
        for b in range(BPC)
    ]
    acc = [const_pool.tile([128, NCH, IMG], F32, tag=f"acc{b}", name=f"acc{b}") for b in range(BPC)]

    # ---------------- prep phase ----------------
    with (
        tc.tile_pool(name="prep_sb", bufs=4) as prep_sb,
        tc.tile_pool(name="prep_ps", bufs=4, space="PSUM") as prep_ps,
    ):
        # W3T[kd][:, ke*128:+128] = W3[ke-chunk, kd-chunk].T
        for ke in range(NCH):
            row = prep_sb.tile([128, DIM], F32, tag="w3row")
            nc.sync.dma_start(out=row[:], in_=w3_d[ke * 128 : (ke + 1) * 128, :])
            for kd in range(NCH):
                ps = prep_ps.tile([128, 128], F32, tag="tr")
                nc.tensor.transpose(
                    ps[:], row[:, kd * 128 : (kd + 1) * 128], ident[:]
                )
                nc.scalar.copy(w3t[kd][:, ke * 128 : (ke + 1) * 128], ps[:])

        # W1T[wc][0:wsz, kd*128:+128] = W1[kd-chunk, w-chunk].T
        for kd in range(NCH):
            row = prep_sb.tile([128, WDIM], F32, tag="w1row")
            nc.sync.dma_start(out=row[:], in_=w1_d[kd * 128 : (kd + 1) * 128, :])
            for wc, (woff, wsz) in enumerate(WCH):
                ps = prep_ps.tile([128, 128], F32, tag="tr")
                nc.tensor.transpose(
                    ps[:wsz, :], row[:, woff : woff + wsz], ident[:]
                )
                nc.scalar.copy(w1t[wc][:wsz, kd * 128 : (kd + 1) * 128], ps[:wsz, :])

        for b in range(BPC):
            # tokenT
            for sc, (soff, ssz) in enumerate(SCH):
                ts = prep_sb.tile([128, DIM], F32, tag="tokrow")
                nc.sync.dma_start(
                    out=ts[:ssz, :], in_=token_d[b, soff : soff + ssz, :]
                )
                for kd in range(NCH):
                    ps = prep_ps.tile([128, 128], F32, tag="tr")
                    nc.tensor.transpose(
                        ps[:, :ssz],
                        ts[:ssz, kd * 128 : (kd + 1) * 128],
                        ident[:ssz, :ssz],
                    )
                    nc.scalar.copy(tokT[b][kd][:, soff : soff + ssz], ps[:, :ssz])

            # lgT then labT = (lg @ W1.T).T
            lgrow = prep_sb.tile([128, WDIM], F32, tag="lgrow")
            nc.sync.dma_start(out=lgrow[:CLS, :], in_=lg_d[b, :, :])
            lgt = []
            for wc, (woff, wsz) in enumerate(WCH):
                ps = prep_ps.tile([128, 128], F32, tag="tr")
                nc.tensor.transpose(
                    ps[:wsz, :CLS],
                    lgrow[:CLS, woff : woff + wsz],
                    ident[:CLS, :CLS],
                )
                t = prep_sb.tile([128, CLS], F32, tag=f"lgt{wc}")
                nc.scalar.copy(t[:wsz, :], ps[:wsz, :CLS])
                lgt.append(t)
            for kd in range(NCH):
                ps = prep_ps.tile([128, CLS], F32, tag="lab")
                for wc, (woff, wsz) in enumerate(WCH):
                    nc.tensor.matmul(
                        ps[:],
                        w1t[wc][:wsz, kd * 128 : (kd + 1) * 128],
                        lgt[wc][:wsz, :],
                        start=(wc == 0),
                        stop=(wc == len(WCH) - 1),
                    )
                nc.scalar.copy(labT[b][kd][:], ps[:])

            nc.vector.memset(acc[b][:], 0.0)

    # ---------------- main loop ----------------
    with (
        tc.tile_pool(name="stage", bufs=2) as stage_pool,
        tc.tile_pool(name="prod", bufs=2) as prod_pool,
        tc.tile_pool(name="epool", bufs=2) as e_pool,
        tc.tile_pool(name="zpool", bufs=3) as z_pool,
        tc.tile_pool(name="feat", bufs=2, space="PSUM") as feat_pool,
    ):
        for b in range(BPC):
            for p in range(CLS // 2):
                cA, cB = 2 * p, 2 * p + 1
                stage = stage_pool.tile([128, NCH, 2 * IMG], F32, tag="stage")
                for m in range(NCH):
                    nc.vector.tensor_scalar_mul(
                        stage[:, m, 0:IMG], tokT[b][m][:], labT[b][m][:, cA : cA + 1]
                    )
                    nc.vector.tensor_scalar_mul(
                        stage[:, m, IMG : 2 * IMG],
                        tokT[b][m][:],
                        labT[b][m][:, cB : cB + 1],
                    )
                prod = prod_pool.tile([128, NCH, 2 * IMG], mm_dt, tag="prod")
                for m in range(NCH):
                    nc.scalar.activation(prod[:, m, :], stage[:, m, :], TS.Tanh)

                feat = feat_pool.tile([128, NCH, 512], F32, tag="feat")
                for m in range(NCH):
                    for k in range(NCH):
                        nc.tensor.matmul(
                            feat[:, m, 0 : 2 * IMG],
                            w3t[k][:, m * 128 : (m + 1) * 128],
                            prod[:, k, :],
                            start=(k == 0),
                            stop=(k == NCH - 1),
                        )

                E = e_pool.tile([128, NCH, 2 * IMG], F32, tag="E")
                Z = z_pool.tile([128, 8], F32, tag="Z")
                for m in range(NCH):
                    nc.scalar.activation(
                        E[:, m, 0:IMG],
                        feat[:, m, 0:IMG],
                        TS.Exp,
                        accum_out=Z[:, 2 * m : 2 * m + 1],
                    )
                    nc.scalar.activation(
                        E[:, m, IMG : 2 * IMG],
                        feat[:, m, IMG : 2 * IMG],
                        TS.Exp,
                        accum_out=Z[:, 2 * m + 1 : 2 * m + 2],
                    )
                R = z_pool.tile([128, 8], F32, tag="R")
                nc.vector.reciprocal(R[:], Z[:])
                for m in range(NCH):
                    nc.vector.scalar_tensor_tensor(
                        acc[b][:, m, :],
                        E[:, m, 0:IMG],
                        R[:, 2 * m : 2 * m + 1],
                        acc[b][:, m, :],
                        AO.mult,
                        AO.add,
                    )
                    nc.vector.scalar_tensor_tensor(
                        acc[b][:, m, :],
                        E[:, m, IMG : 2 * IMG],
                        R[:, 2 * m + 1 : 2 * m + 2],
                        acc[b][:, m, :],
                        AO.mult,
                        AO.add,
                    )

    # ---------------- final phase ----------------
    with (
        tc.tile_pool(name="fin_sb", bufs=2) as fin_sb,
        tc.tile_pool(name="fin_ps", bufs=4, space="PSUM") as fin_ps,
    ):
        for b in range(BPC):
            outT = fin_sb.tile([128, NCH, IMG], F32, tag="outT")
            for m in range(NCH):
                nc.vector.tensor_tensor(
                    outT[:, m, :],
                    tokT[b][m][:],
                    acc[b][:, m, :],
                    mybir.AluOpType.mult,
                )
            for sc, (soff, ssz) in enumerate(SCH):
                outs = fin_sb.tile([128, DIM], F32, tag=f"outs{sc}")
                for m in range(NCH):
                    ps = fin_ps.tile([128, 128], F32, tag="tr")
                    nc.tensor.transpose(
                        ps[:ssz, :], outT[:, m, soff : soff + ssz], ident[:]
                    )
                    nc.scalar.copy(outs[:ssz, m * 128 : (m + 1) * 128], ps[:ssz, :])
                nc.sync.dma_start(
                    out=out_d[b, soff : soff + ssz, :], in_=outs[:ssz, :]
                )


def emit_body_v2(tc, nc, token_d, lg_d, w1_d, w3_d, out_d, zdve_frac=0.64):
    with tc.tile_pool(name="const", bufs=1) as const_pool:
        _emit_v2_inner(tc, nc, token_d, lg_d, w1_d, w3_d, out_d, const_pool, zdve_frac)


def _emit_v2_inner(tc, nc, token_d, lg_d, w1_d, w3_d, out_d, cp, zdve_frac):
    TS = mybir.ActivationFunctionType
    AO = mybir.AluOpType
    BF16 = mybir.dt.bfloat16
    X = mybir.AxisListType.X

    ident = cp.tile([128, 128], F32)
    masks.make_identity(nc, ident[:])
    ident_bf = cp.tile([128, 128], BF16)
    masks.make_identity(nc, ident_bf[:])

    w3t = [cp.tile([128, DIM], BF16, tag=f"w3t{k}", name=f"w3t{k}") for k in range(NCH)]
    w1t = [cp.tile([128, DIM], F32, tag=f"w1t{w}", name=f"w1t{w}") for w in range(len(WCH))]
    tokT = [
        [cp.tile([128, IMG], F32, tag=f"tokT{b}_{k}", name=f"tokT{b}_{k}") for k in range(NCH)]
        for b in range(BPC)
    ]
    tokB = [
        [cp.tile([128, IMG], BF16, tag=f"tokB{b}_{k}", name=f"tokB{b}_{k}") for k in range(NCH)]
        for b in range(BPC)
    ]
    labT = [
        [cp.tile([128, CLS], F32, tag=f"labT{b}_{k}", name=f"labT{b}_{k}") for k in range(NCH)]
        for b in range(BPC)
    ]

    # ---------------- prep ----------------
    with (
        tc.tile_pool(name="prep_sb", bufs=4) as prep_sb,
        tc.tile_pool(name="prep_ps", bufs=4, space="PSUM") as prep_ps,
    ):
        _cp_i = [0]

        def pcopy(dst, src_):
            eng = nc.scalar if _cp_i[0] % 2 == 0 else nc.vector
            _cp_i[0] += 1
            if eng is nc.scalar:
                eng.copy(dst, src_)
            else:
                eng.tensor_copy(dst, src_)

        for ke in range(NCH):
            row = prep_sb.tile([128, DIM], F32, tag="w3row")
            nc.sync.dma_start(out=row[:], in_=w3_d[ke * 128 : (ke + 1) * 128, :])
            for kd in range(NCH):
                ps = prep_ps.tile([128, 128], F32, tag="tr")
                nc.tensor.transpose(ps[:], row[:, kd * 128 : (kd + 1) * 128], ident[:])
                pcopy(w3t[kd][:, ke * 128 : (ke + 1) * 128], ps[:])

        for kd in range(NCH):
            row = prep_sb.tile([128, WDIM], F32, tag="w1row")
            nc.sync.dma_start(out=row[:], in_=w1_d[kd * 128 : (kd + 1) * 128, :])
            for wc, (woff, wsz) in enumerate(WCH):
                ps = prep_ps.tile([128, 128], F32, tag="tr")
                nc.tensor.transpose(ps[:wsz, :], row[:, woff : woff + wsz], ident[:])
                pcopy(w1t[wc][:wsz, kd * 128 : (kd + 1) * 128], ps[:wsz, :])

        for b in range(BPC):
            for sc, (soff, ssz) in enumerate(SCH):
                ts_ = prep_sb.tile([128, DIM], F32, tag="tokrow")
                nc.sync.dma_start(out=ts_[:ssz, :], in_=token_d[b, soff : soff + ssz, :])
                for kd in range(NCH):
                    ps = prep_ps.tile([128, 128], F32, tag="tr")
                    nc.tensor.transpose(
                        ps[:, :ssz],
                        ts_[:ssz, kd * 128 : (kd + 1) * 128],
                        ident[:ssz, :ssz],
                    )
                    pcopy(tokT[b][kd][:, soff : soff + ssz], ps[:, :ssz])
            for kd in range(NCH):
                nc.vector.tensor_copy(tokB[b][kd][:], tokT[b][kd][:])

            lgrow = prep_sb.tile([128, WDIM], F32, tag="lgrow")
            nc.sync.dma_start(out=lgrow[:CLS, :], in_=lg_d[b, :, :])
            lgt = []
            for wc, (woff, wsz) in enumerate(WCH):
                ps = prep_ps.tile([128, 128], F32, tag="tr")
                nc.tensor.transpose(
                    ps[:wsz, :CLS], lgrow[:CLS, woff : woff + wsz], ident[:CLS, :CLS]
                )
                t = prep_sb.tile([128, CLS], F32, tag=f"lgt{wc}")
                pcopy(t[:wsz, :], ps[:wsz, :CLS])
                lgt.append(t)
            for kd in range(NCH):
                ps = prep_ps.tile([128, CLS], F32, tag="lab")
                for wc, (woff, wsz) in enumerate(WCH):
                    nc.tensor.matmul(
                        ps[:],
                        w1t[wc][:wsz, kd * 128 : (kd + 1) * 128],
                        lgt[wc][:wsz, :],
                        start=(wc == 0),
                        stop=(wc == len(WCH) - 1),
                    )
                pcopy(labT[b][kd][:], ps[:])

    # ---------------- main ----------------
    NPAIR = CLS // 2  # 40 pairs per batch
    OCT = 4  # pairs per octet group (8 classes)
    with tc.tile_pool(name="accps", bufs=1, space="PSUM") as acc_pool, tc.tile_pool(
        name="fin_sb", bufs=2
    ) as fin_sb:
        # acc slot for chunk m: bank m//2, free offset (m%2)*256, length IMG
        def acc_slot(acc_ps, m):
            return acc_ps[:, m // 2, (m % 2) * 256 : (m % 2) * 256 + IMG]

        for b in range(BPC):
            acc_ps = acc_pool.tile([128, 2, 512], F32, tag="acc", name=f"accps{b}")
            with (
                tc.tile_pool(name="stage", bufs=2) as stage_pool,
                tc.tile_pool(name="prod", bufs=2) as prod_pool,
                tc.tile_pool(name="epool", bufs=3) as e_pool,
                tc.tile_pool(name="zpool", bufs=4) as z_pool,
                tc.tile_pool(name="feat", bufs=3, space="PSUM") as feat_pool,
            ):
                pending = None  # (p, E) whose identity-accumulate is deferred

                def flush_pending():
                    nonlocal pending
                    if pending is None:
                        return
                    pp, pE = pending
                    for m in range(NCH):
                        for j in range(2):
                            # start=True clears the WHOLE bank, so only the
                            # first matmul into each bank may set it
                            nc.tensor.matmul(
                                acc_slot(acc_ps, m),
                                ident_bf[:],
                                pE[:, m, j * IMG : (j + 1) * IMG],
                                start=(pp == 0 and j == 0 and m % 2 == 0),
                                stop=(pp == NPAIR - 1 and j == 1),
                                skip_group_check=True,
                            )
                    pending = None

                for g in range(NPAIR // OCT):  # octet groups
                    stage = stage_pool.tile([128, NCH, OCT * 2 * IMG], BF16, tag="stage")
                    prod = prod_pool.tile([128, NCH, OCT * 2 * IMG], BF16, tag="prod")
                    for m in range(NCH):
                        for j in range(2 * OCT):
                            c = g * 2 * OCT + j
                            nc.vector.tensor_scalar_mul(
                                stage[:, m, j * IMG : (j + 1) * IMG],
                                tokB[b][m][:],
                                labT[b][m][:, c : c + 1],
                            )
                        nc.scalar.activation(prod[:, m, :], stage[:, m, :], TS.Tanh)
                    for pj in range(OCT):
                        p = g * OCT + pj
                        kf = int(round(zdve_frac * NPAIR))
                        z_dve = ((p + 1) * kf) // NPAIR > (p * kf) // NPAIR
                        E = e_pool.tile([128, NCH, 2 * IMG], BF16, tag="E", name=f"E{b}_{p}")
                        Z = z_pool.tile([128, 8], F32, tag="Z", name=f"Z{b}_{p}")
                        R = z_pool.tile([128, 8], F32, tag="R", name=f"R{b}_{p}")
                        for half in range(2):
                            feat = feat_pool.tile(
                                [128, 2, 512], F32, tag="feat", name=f"feat{b}_{p}_{half}"
                            )
                            for mi in range(2):
                                m = half * 2 + mi
                                for k in range(NCH):
                                    nc.tensor.matmul(
                                        feat[:, mi, 0 : 2 * IMG],
                                        w3t[k][:, m * 128 : (m + 1) * 128],
                                        prod[:, k, pj * 2 * IMG : (pj + 1) * 2 * IMG],
                                        start=(k == 0),
                                        stop=(k == NCH - 1),
                                    )
                            if half == 1:
                                # previous pair's deferred accumulate rides here,
                                # after this pair's main matmuls are queued
                                flush_pending()
                            if z_dve:
                                nc.scalar.activation(
                                    E[:, half * 2 : half * 2 + 2, :],
                                    feat[:, :, 0 : 2 * IMG],
                                    TS.Exp,
                                )
                            else:
                                for mi in range(2):
                                    m = half * 2 + mi
                                    for j in range(2):
                                        nc.scalar.activation(
                                            E[:, m, j * IMG : (j + 1) * IMG],
                                            feat[:, mi, j * IMG : (j + 1) * IMG],
                                            TS.Exp,
                                            accum_out=Z[:, 2 * m + j : 2 * m + j + 1],
                                        )
                        if z_dve:
                            nc.vector.tensor_reduce(
                                Z[:],
                                E[:].rearrange("p m (j s) -> p m j s", j=2),
                                axis=X,
                                op=AO.add,
                            )
                        nc.vector.reciprocal(R[:], Z[:])
                        # scale E in place; identity-accumulate is deferred one pair
                        for m in range(NCH):
                            for j in range(2):
                                nc.vector.tensor_scalar_mul(
                                    E[:, m, j * IMG : (j + 1) * IMG],
                                    E[:, m, j * IMG : (j + 1) * IMG],
                                    R[:, 2 * m + j : 2 * m + j + 1],
                                )
                        pending = (p, E)
                flush_pending()

            # ---------------- final for this batch ----------------
            with tc.tile_pool(name="fin_ps", bufs=4, space="PSUM") as fin_ps:
                outT = fin_sb.tile([128, NCH, IMG], F32, tag="outT", name=f"outT{b}")
                for m in range(NCH):
                    nc.vector.tensor_tensor(
                        outT[:, m, :], tokT[b][m][:], acc_slot(acc_ps, m), AO.mult
                    )
                for sc, (soff, ssz) in enumerate(SCH):
                    outs = fin_sb.tile([128, DIM], F32, tag=f"outs{sc}", name=f"outs{b}_{sc}")
                    for m in range(NCH):
                        ps = fin_ps.tile([128, 128], F32, tag="tr")
                        nc.tensor.transpose(
                            ps[:ssz, :], outT[:, m, soff : soff + ssz], ident[:]
                        )
                        nc.scalar.copy(outs[:ssz, m * 128 : (m + 1) * 128], ps[:ssz, :])
                    nc.sync.dma_start(out=out_d[b, soff : soff + ssz, :], in_=outs[:ssz, :])


_NC_CACHE = {}


def _get_nc(mm_dt=MM_DT, variant="v2", zdve_frac=0.64):
    key = (str(mm_dt), variant, zdve_frac)
    if key not in _NC_CACHE:
        _NC_CACHE[key] = build_kernel(mm_dt, variant, zdve_frac)
    return _NC_CACHE[key]


def run(inputs, trace=False, mm_dt=MM_DT, variant="v2", zdve_frac=0.64):
    token = np.ascontiguousarray(np.asarray(inputs["tokenFeaturemap"], np.float32))
    lg = np.ascontiguousarray(np.asarray(inputs["labelGraphfeatures"], np.float32))
    w1 = np.ascontiguousarray(np.asarray(inputs["W1"], np.float32))
    w3 = np.ascontiguousarray(np.asarray(inputs["W3"], np.float32))
    nc = _get_nc(mm_dt, variant, zdve_frac)
    in_maps = [
        {
            "token": token[i * BPC : (i + 1) * BPC],
            "lg": lg[i * BPC : (i + 1) * BPC],
            "W1": w1,
            "W3": w3,
        }
        for i in range(N_CORES)
    ]
    res = run_bass_kernel_spmd(nc, in_maps, list(range(N_CORES)), trace=trace)
    out = np.concatenate([res.results[i]["out"] for i in range(N_CORES)], axis=0)
    return out, res


def kernel(**inputs) -> np.ndarray:
    out, _ = run(inputs)
    return out


# revision 17
# speedup vs baseline: 2.1797x; 1.0776x over previous
import os
import sys

for _p in ("/opt/trn_rl_repo", "/root/.axon_site/_ro/trn_rl_repo"):
    if os.path.isdir(_p) and _p not in sys.path:
        sys.path.append(_p)

import numpy as np
import concourse.bass as bass
import concourse.mybir as mybir
import concourse.tile as tile
from concourse import masks
from concourse.bass_utils import run_bass_kernel_spmd

# Problem shapes (hardcoded per contract)
BS, IMG, CLS, DIM, WDIM = 16, 196, 80, 512, 300
N_CORES = 8
BPC = BS // N_CORES  # batches per core
NCH = DIM // 128  # 4 d-chunks of 128
SCH = [(0, 128), (128, 68)]  # s-chunks (offset, size) for IMG=196
WCH = [(0, 128), (128, 128), (256, 44)]  # w-chunks for WDIM=300

F32 = mybir.dt.float32
# dtype used for the fc3 matmul operands (W3T weights and prodT moving data)
MM_DT = mybir.dt.float32


def split_multi_waits(nc):
    """This walrus build accepts a single sync-wait per instruction on the
    CTRL encodings; split extra waits into single-wait NoOps on the same
    engine immediately before the instruction."""
    k = 0
    for f in nc.m.functions:
        for bb in f.blocks:
            il = bb.instructions
            i = 0
            while i < len(il):
                ins = il[i]
                si = ins.sync_info
                if si is not None and len(si.on_wait) > 1:
                    waits = list(si.on_wait)
                    for w in waits[:-1]:
                        nop = mybir.InstNoOp(name=f"waitsplit_{k}", ins=[], outs=[])
                        k += 1
                        nop.engine = ins.engine
                        nop.sync_info = mybir.SyncInfo(on_wait=[w], on_update=[])
                        il.insert(i, nop)
                        i += 1
                    ins.sync_info = mybir.SyncInfo(
                        on_wait=[waits[-1]], on_update=list(si.on_update)
                    )
                i += 1
    return k


def build_kernel(mm_dt=MM_DT, variant="v2", zdve_frac=0.64):
    nc = bass.Bass("TRN2", target_bir_lowering=False, debug=False, num_devices=N_CORES)
    token_d = nc.dram_tensor("token", [BPC, IMG, DIM], F32, kind="ExternalInput").ap()
    lg_d = nc.dram_tensor("lg", [BPC, CLS, WDIM], F32, kind="ExternalInput").ap()
    w1_d = nc.dram_tensor("W1", [DIM, WDIM], F32, kind="ExternalInput").ap()
    w3_d = nc.dram_tensor("W3", [DIM, DIM], F32, kind="ExternalInput").ap()
    out_d = nc.dram_tensor("out", [BPC, IMG, DIM], F32, kind="ExternalOutput").ap()

    with tile.TileContext(nc) as tc:
        if variant == "v1":
            emit_body(tc, nc, token_d, lg_d, w1_d, w3_d, out_d, mm_dt)
        else:
            emit_body_v2(tc, nc, token_d, lg_d, w1_d, w3_d, out_d, zdve_frac)
    split_multi_waits(nc)
    return nc


def emit_body(tc, nc, token_d, lg_d, w1_d, w3_d, out_d, mm_dt):
    TS = mybir.ActivationFunctionType
    AO = mybir.AluOpType

    with tc.tile_pool(name="const", bufs=1) as const_pool:
        _emit_body_inner(tc, nc, token_d, lg_d, w1_d, w3_d, out_d, mm_dt, const_pool)


def _emit_body_inner(tc, nc, token_d, lg_d, w1_d, w3_d, out_d, mm_dt, const_pool):
    TS = mybir.ActivationFunctionType
    AO = mybir.AluOpType

    ident = const_pool.tile([128, 128], F32)
    masks.make_identity(nc, ident[:])

    # Persistent SBUF tensors
    w3t = [const_pool.tile([128, DIM], mm_dt, tag=f"w3t{k}", name=f"w3t{k}") for k in range(NCH)]
    w1t = [const_pool.tile([128, DIM], F32, tag=f"w1t{w}", name=f"w1t{w}") for w in range(len(WCH))]
    tokT = [
        [const_pool.tile([128, IMG], F32, tag=f"tokT{b}_{k}", name=f"tokT{b}_{k}") for k in range(NCH)]
        for b in range(BPC)
    ]
    labT = [
        [const_pool.tile([128, CLS], F32, tag=f"labT{b}_{k}", name=f"labT{b}_{k}") for k in range(NCH)]
        for b in range(BPC)
    ]
    acc = [const_pool.tile([128, NCH, IMG], F32, tag=f"acc{b}", name=f"acc{b}") for b in range(BPC)]

    # ---------------- prep phase ----------------
    with (
        tc.tile_pool(name="prep_sb", bufs=4) as prep_sb,
        tc.tile_pool(name="prep_ps", bufs=4, space="PSUM") as prep_ps,
    ):
        # W3T[kd][:, ke*128:+128] = W3[ke-chunk, kd-chunk].T
        for ke in range(NCH):
            row = prep_sb.tile([128, DIM], F32, tag="w3row")
            nc.sync.dma_start(out=row[:], in_=w3_d[ke * 128 : (ke + 1) * 128, :])
            for kd in range(NCH):
                ps = prep_ps.tile([128, 128], F32, tag="tr")
                nc.tensor.transpose(
                    ps[:], row[:, kd * 128 : (kd + 1) * 128], ident[:]
                )
                nc.scalar.copy(w3t[kd][:, ke * 128 : (ke + 1) * 128], ps[:])

        # W1T[wc][0:wsz, kd*128:+128] = W1[kd-chunk, w-chunk].T
        for kd in range(NCH):
            row = prep_sb.tile([128, WDIM], F32, tag="w1row")
            nc.sync.dma_start(out=row[:], in_=w1_d[kd * 128 : (kd + 1) * 128, :])
            for wc, (woff, wsz) in enumerate(WCH):
                ps = prep_ps.tile([128, 128], F32, tag="tr")
                nc.tensor.transpose(
                    ps[:wsz, :], row[:, woff : woff + wsz], ident[:]
                )
                nc.scalar.copy(w1t[wc][:wsz, kd * 128 : (kd + 1) * 128], ps[:wsz, :])

        for b in range(BPC):
            # tokenT
            for sc, (soff, ssz) in enumerate(SCH):
                ts = prep_sb.tile([128, DIM], F32, tag="tokrow")
                nc.sync.dma_start(
                    out=ts[:ssz, :], in_=token_d[b, soff : soff + ssz, :]
                )
                for kd in range(NCH):
                    ps = prep_ps.tile([128, 128], F32, tag="tr")
                    nc.tensor.transpose(
                        ps[:, :ssz],
                        ts[:ssz, kd * 128 : (kd + 1) * 128],
                        ident[:ssz, :ssz],
                    )
                    nc.scalar.copy(tokT[b][kd][:, soff : soff + ssz], ps[:, :ssz])

            # lgT then labT = (lg @ W1.T).T
            lgrow = prep_sb.tile([128, WDIM], F32, tag="lgrow")
            nc.sync.dma_start(out=lgrow[:CLS, :], in_=lg_d[b, :, :])
            lgt = []
            for wc, (woff, wsz) in enumerate(WCH):
                ps = prep_ps.tile([128, 128], F32, tag="tr")
                nc.tensor.transpose(
                    ps[:wsz, :CLS],
                    lgrow[:CLS, woff : woff + wsz],
                    ident[:CLS, :CLS],
                )
                t = prep_sb.tile([128, CLS], F32, tag=f"lgt{wc}")
                nc.scalar.copy(t[:wsz, :], ps[:wsz, :CLS])
                lgt.append(t)
            for kd in range(NCH):
                ps = prep_ps.tile([128, CLS], F32, tag="lab")
                for wc, (woff, wsz) in enumerate(WCH):
                    nc.tensor.matmul(
                        ps[:],
                        w1t[wc][:wsz, kd * 128 : (kd + 1) * 128],
                        lgt[wc][:wsz, :],
                        start=(wc == 0),
                        stop=(wc == len(WCH) - 1),
                    )
                nc.scalar.copy(labT[b][kd][:], ps[:])

            nc.vector.memset(acc[b][:], 0.0)

    # ---------------- main loop ----------------
    with (
        tc.tile_pool(name="stage", bufs=2) as stage_pool,
        tc.tile_pool(name="prod", bufs=2) as prod_pool,
        tc.tile_pool(name="epool", bufs=2) as e_pool,
        tc.tile_pool(name="zpool", bufs=3) as z_pool,
        tc.tile_pool(name="feat", bufs=2, space="PSUM") as feat_pool,
    ):
        for b in range(BPC):
            for p in range(CLS // 2):
                cA, cB = 2 * p, 2 * p + 1
                stage = stage_pool.tile([128, NCH, 2 * IMG], F32, tag="stage")
                for m in range(NCH):
                    nc.vector.tensor_scalar_mul(
                        stage[:, m, 0:IMG], tokT[b][m][:], labT[b][m][:, cA : cA + 1]
                    )
                    nc.vector.tensor_scalar_mul(
                        stage[:, m, IMG : 2 * IMG],
                        tokT[b][m][:],
                        labT[b][m][:, cB : cB + 1],
                    )
                prod = prod_pool.tile([128, NCH, 2 * IMG], mm_dt, tag="prod")
                for m in range(NCH):
                    nc.scalar.activation(prod[:, m, :], stage[:, m, :], TS.Tanh)

                feat = feat_pool.tile([128, NCH, 512], F32, tag="feat")
                for m in range(NCH):
                    for k in range(NCH):
                        nc.tensor.matmul(
                            feat[:, m, 0 : 2 * IMG],
                            w3t[k][:, m * 128 : (m + 1) * 128],
                            prod[:, k, :],
                            start=(k == 0),
                            stop=(k == NCH - 1),
                        )

                E = e_pool.tile([128, NCH, 2 * IMG], F32, tag="E")
                Z = z_pool.tile([128, 8], F32, tag="Z")
                for m in range(NCH):
                    nc.scalar.activation(
                        E[:, m, 0:IMG],
                        feat[:, m, 0:IMG],
                        TS.Exp,
                        accum_out=Z[:, 2 * m : 2 * m + 1],
                    )
                    nc.scalar.activation(
                        E[:, m, IMG : 2 * IMG],
                        feat[:, m, IMG : 2 * IMG],
                        TS.Exp,
                        accum_out=Z[:, 2 * m + 1 : 2 * m + 2],
                    )
                R = z_pool.tile([128, 8], F32, tag="R")
                nc.vector.reciprocal(R[:], Z[:])
                for m in range(NCH):
                    nc.vector.scalar_tensor_tensor(
                        acc[b][:, m, :],
                        E[:, m, 0:IMG],
                        R[:, 2 * m : 2 * m + 1],
                        acc[b][:, m, :],
                        AO.mult,
                        AO.add,
                    )
                    nc.vector.scalar_tensor_tensor(
                        acc[b][:, m, :],
                        E[:, m, IMG : 2 * IMG],
                        R[:, 2 * m + 1 : 2 * m + 2],
                        acc[b][:, m, :],
                        AO.mult,
                        AO.add,
                    )

    # ---------------- final phase ----------------
    with (
        tc.tile_pool(name="fin_sb", bufs=2) as fin_sb,
        tc.tile_pool(name="fin_ps", bufs=4, space="PSUM") as fin_ps,
    ):
        for b in range(BPC):
            outT = fin_sb.tile([128, NCH, IMG], F32, tag="outT")
            for m in range(NCH):
                nc.vector.tensor_tensor(
                    outT[:, m, :],
                    tokT[b][m][:],
                    acc[b][:, m, :],
                    mybir.AluOpType.mult,
                )
            for sc, (soff, ssz) in enumerate(SCH):
                outs = fin_sb.tile([128, DIM], F32, tag=f"outs{sc}")
                for m in range(NCH):
                    ps = fin_ps.tile([128, 128], F32, tag="tr")
                    nc.tensor.transpose(
                        ps[:ssz, :], outT[:, m, soff : soff + ssz], ident[:]
                    )
                    nc.scalar.copy(outs[:ssz, m * 128 : (m + 1) * 128], ps[:ssz, :])
                nc.sync.dma_start(
                    out=out_d[b, soff : soff + ssz, :], in_=outs[:ssz, :]
                )


def emit_body_v2(tc, nc, token_d, lg_d, w1_d, w3_d, out_d, zdve_frac=0.64):
    with tc.tile_pool(name="const", bufs=1) as const_pool:
        _emit_v2_inner(tc, nc, token_d, lg_d, w1_d, w3_d, out_d, const_pool, zdve_frac)


def _emit_v2_inner(tc, nc, token_d, lg_d, w1_d, w3_d, out_d, cp, zdve_frac):
    TS = mybir.ActivationFunctionType
    AO = mybir.AluOpType
    BF16 = mybir.dt.bfloat16
    X = mybir.AxisListType.X

    ident = cp.tile([128, 128], F32)
    masks.make_identity(nc, ident[:])
    ident_bf = cp.tile([128, 128], BF16)
    masks.make_identity(nc, ident_bf[:])

    w3t = [cp.tile([128, DIM], BF16, tag=f"w3t{k}", name=f"w3t{k}") for k in range(NCH)]
    w1t = [cp.tile([128, DIM], F32, tag=f"w1t{w}", name=f"w1t{w}") for w in range(len(WCH))]
    tokT = [
        [cp.tile([128, IMG], F32, tag=f"tokT{b}_{k}", name=f"tokT{b}_{k}") for k in range(NCH)]
        for b in range(BPC)
    ]
    tokB = [
        [cp.tile([128, IMG], BF16, tag=f"tokB{b}_{k}", name=f"tokB{b}_{k}") for k in range(NCH)]
        for b in range(BPC)
    ]
    labT = [
        [cp.tile([128, CLS], F32, tag=f"labT{b}_{k}", name=f"labT{b}_{k}") for k in range(NCH)]
        for b in range(BPC)
    ]

    # ---------------- prep ----------------
    with (
        tc.tile_pool(name="prep_sb", bufs=4) as prep_sb,
        tc.tile_pool(name="prep_ps", bufs=4, space="PSUM") as prep_ps,
    ):
        _cp_i = [0]

        def pcopy(dst, src_):
            eng = nc.scalar if _cp_i[0] % 2 == 0 else nc.vector
            _cp_i[0] += 1
            if eng is nc.scalar:
                eng.copy(dst, src_)
            else:
                eng.tensor_copy(dst, src_)

        for ke in range(NCH):
            row = prep_sb.tile([128, DIM], F32, tag="w3row")
            nc.sync.dma_start(out=row[:], in_=w3_d[ke * 128 : (ke + 1) * 128, :])
            for kd in range(NCH):
                ps = prep_ps.tile([128, 128], F32, tag="tr")
                nc.tensor.transpose(ps[:], row[:, kd * 128 : (kd + 1) * 128], ident[:])
                pcopy(w3t[kd][:, ke * 128 : (ke + 1) * 128], ps[:])

        for kd in range(NCH):
            row = prep_sb.tile([128, WDIM], F32, tag="w1row")
            nc.sync.dma_start(out=row[:], in_=w1_d[kd * 128 : (kd + 1) * 128, :])
            for wc, (woff, wsz) in enumerate(WCH):
                ps = prep_ps.tile([128, 128], F32, tag="tr")
                nc.tensor.transpose(ps[:wsz, :], row[:, woff : woff + wsz], ident[:])
                pcopy(w1t[wc][:wsz, kd * 128 : (kd + 1) * 128], ps[:wsz, :])

        for b in range(BPC):
            for sc, (soff, ssz) in enumerate(SCH):
                ts_ = prep_sb.tile([128, DIM], F32, tag="tokrow")
                nc.sync.dma_start(out=ts_[:ssz, :], in_=token_d[b, soff : soff + ssz, :])
                for kd in range(NCH):
                    ps = prep_ps.tile([128, 128], F32, tag="tr")
                    nc.tensor.transpose(
                        ps[:, :ssz],
                        ts_[:ssz, kd * 128 : (kd + 1) * 128],
                        ident[:ssz, :ssz],
                    )
                    pcopy(tokT[b][kd][:, soff : soff + ssz], ps[:, :ssz])
            for kd in range(NCH):
                nc.vector.tensor_copy(tokB[b][kd][:], tokT[b][kd][:])

            lgrow = prep_sb.tile([128, WDIM], F32, tag="lgrow")
            nc.sync.dma_start(out=lgrow[:CLS, :], in_=lg_d[b, :, :])
            lgt = []
            for wc, (woff, wsz) in enumerate(WCH):
                ps = prep_ps.tile([128, 128], F32, tag="tr")
                nc.tensor.transpose(
                    ps[:wsz, :CLS], lgrow[:CLS, woff : woff + wsz], ident[:CLS, :CLS]
                )
                t = prep_sb.tile([128, CLS], F32, tag=f"lgt{wc}")
                pcopy(t[:wsz, :], ps[:wsz, :CLS])
                lgt.append(t)
            for kd in range(NCH):
                ps = prep_ps.tile([128, CLS], F32, tag="lab")
                for wc, (woff, wsz) in enumerate(WCH):
                    nc.tensor.matmul(
                        ps[:],
                        w1t[wc][:wsz, kd * 128 : (kd + 1) * 128],
                        lgt[wc][:wsz, :],
                        start=(wc == 0),
                        stop=(wc == len(WCH) - 1),
                    )
                pcopy(labT[b][kd][:], ps[:])

    # ---------------- main ----------------
    NPAIR = CLS // 2  # 40 pairs per batch
    OCT = 4  # pairs per octet group (8 classes)
    with tc.tile_pool(name="accps", bufs=1, space="PSUM") as acc_pool, tc.tile_pool(
        name="fin_sb", bufs=2
    ) as fin_sb:
        # acc slot for chunk m: bank m//2, free offset (m%2)*256, length IMG
        def acc_slot(acc_ps, m):
            return acc_ps[:, m // 2, (m % 2) * 256 : (m % 2) * 256 + IMG]

        for b in range(BPC):
            acc_ps = acc_pool.tile([128, 2, 512], F32, tag="acc", name=f"accps{b}")
            with (
                tc.tile_pool(name="stage", bufs=2) as stage_pool,
                tc.tile_pool(name="prod", bufs=2) as prod_pool,
                tc.tile_pool(name="epool", bufs=3) as e_pool,
                tc.tile_pool(name="zpool", bufs=4) as z_pool,
                tc.tile_pool(name="feat", bufs=3, space="PSUM") as feat_pool,
            ):
                pending = None  # (p, E, diag) whose diag-accumulate is deferred

                def flush_pending():
                    nonlocal pending
                    if pending is None:
                        return
                    pp, pE, pD = pending
                    for m in range(NCH):
                        for j in range(2):
                            # start=True clears the WHOLE bank, so only the
                            # first matmul into each bank may set it
                            nc.tensor.matmul(
                                acc_slot(acc_ps, m),
                                pD[:, 2 * m + j, :],
                                pE[:, m, j * IMG : (j + 1) * IMG],
                                start=(pp == 0 and j == 0 and m % 2 == 0),
                                stop=(pp == NPAIR - 1 and j == 1),
                                skip_group_check=True,
                            )
                    pending = None

                for g in range(NPAIR // OCT):  # octet groups
                    stage = stage_pool.tile([128, NCH, OCT * 2 * IMG], BF16, tag="stage")
                    prod = prod_pool.tile([128, NCH, OCT * 2 * IMG], BF16, tag="prod")
                    for m in range(NCH):
                        for j in range(2 * OCT):
                            c = g * 2 * OCT + j
                            nc.vector.tensor_scalar_mul(
                                stage[:, m, j * IMG : (j + 1) * IMG],
                                tokB[b][m][:],
                                labT[b][m][:, c : c + 1],
                            )
                        nc.scalar.activation(prod[:, m, :], stage[:, m, :], TS.Tanh)
                    for pj in range(OCT):
                        p = g * OCT + pj
                        kf = int(round(zdve_frac * NPAIR))
                        z_dve = ((p + 1) * kf) // NPAIR > (p * kf) // NPAIR
                        E = e_pool.tile([128, NCH, 2 * IMG], BF16, tag="E", name=f"E{b}_{p}")
                        Z = z_pool.tile([128, 8], F32, tag="Z", name=f"Z{b}_{p}")
                        R = z_pool.tile([128, 8], F32, tag="R", name=f"R{b}_{p}")
                        for half in range(2):
                            feat = feat_pool.tile(
                                [128, 2, 512], F32, tag="feat", name=f"feat{b}_{p}_{half}"
                            )
                            for mi in range(2):
                                m = half * 2 + mi
                                for k in range(NCH):
                                    nc.tensor.matmul(
                                        feat[:, mi, 0 : 2 * IMG],
                                        w3t[k][:, m * 128 : (m + 1) * 128],
                                        prod[:, k, pj * 2 * IMG : (pj + 1) * 2 * IMG],
                                        start=(k == 0),
                                        stop=(k == NCH - 1),
                                    )
                            if half == 1:
                                # previous pair's deferred accumulate rides here,
                                # after this pair's main matmuls are queued
                                flush_pending()
                            if z_dve:
                                nc.scalar.activation(
                                    E[:, half * 2 : half * 2 + 2, :],
                                    feat[:, :, 0 : 2 * IMG],
                                    TS.Exp,
                                )
                            else:
                                for mi in range(2):
                                    m = half * 2 + mi
                                    for j in range(2):
                                        nc.scalar.activation(
                                            E[:, m, j * IMG : (j + 1) * IMG],
                                            feat[:, mi, j * IMG : (j + 1) * IMG],
                                            TS.Exp,
                                            accum_out=Z[:, 2 * m + j : 2 * m + j + 1],
                                        )
                        if z_dve:
                            nc.vector.tensor_reduce(
                                Z[:],
                                E[:].rearrange("p m (j s) -> p m j s", j=2),
                                axis=X,
                                op=AO.add,
                            )
                        nc.vector.reciprocal(R[:], Z[:])
                        # build diag(R) for all 8 (chunk, class) slots in one op
                        diag = z_pool.tile(
                            [128, 8, 128], BF16, tag="diag", name=f"diag{b}_{p}"
                        )
                        nc.vector.tensor_tensor(
                            diag[:],
                            ident_bf[:].rearrange("p (o f) -> p o f", o=1).broadcast_to(
                                [128, 8, 128]
                            ),
                            R[:].broadcast_to([128, 8, 128]),
                            AO.mult,
                        )
                        pending = (p, E, diag)
                flush_pending()

            # ---------------- final for this batch ----------------
            with tc.tile_pool(name="fin_ps", bufs=4, space="PSUM") as fin_ps:
                outT = fin_sb.tile([128, NCH, IMG], F32, tag="outT", name=f"outT{b}")
                for m in range(NCH):
                    nc.vector.tensor_tensor(
                        outT[:, m, :], tokT[b][m][:], acc_slot(acc_ps, m), AO.mult
                    )
                for sc, (soff, ssz) in enumerate(SCH):
                    outs = fin_sb.tile([128, DIM], F32, tag=f"outs{sc}", name=f"outs{b}_{sc}")
                    for m in range(NCH):
                        ps = fin_ps.tile([128, 128], F32, tag="tr")
                        nc.tensor.transpose(
                            ps[:ssz, :], outT[:, m, soff : soff + ssz], ident[:]
                        )
                        nc.scalar.copy(outs[:ssz, m * 128 : (m + 1) * 128], ps[:ssz, :])
                    nc.sync.dma_start(out=out_d[b, soff : soff + ssz, :], in_=outs[:ssz, :])


_NC_CACHE = {}


def _get_nc(mm_dt=MM_DT, variant="v2", zdve_frac=0.64):
    key = (str(mm_dt), variant, zdve_frac)
    if key not in _NC_CACHE:
        _NC_CACHE[key] = build_kernel(mm_dt, variant, zdve_frac)
    return _NC_CACHE[key]


def run(inputs, trace=False, mm_dt=MM_DT, variant="v2", zdve_frac=0.64):
    token = np.ascontiguousarray(np.asarray(inputs["tokenFeaturemap"], np.float32))
    lg = np.ascontiguousarray(np.asarray(inputs["labelGraphfeatures"], np.float32))
    w1 = np.ascontiguousarray(np.asarray(inputs["W1"], np.float32))
    w3 = np.ascontiguousarray(np.asarray(inputs["W3"], np.float32))
    nc = _get_nc(mm_dt, variant, zdve_frac)
    in_maps = [
        {
            "token": token[i * BPC : (i + 1) * BPC],
            "lg": lg[i * BPC : (i + 1) * BPC],
            "W1": w1,
            "W3": w3,
        }
        for i in range(N_CORES)
    ]
    res = run_bass_kernel_spmd(nc, in_maps, list(range(N_CORES)), trace=trace)
    out = np.concatenate([res.results[i]["out"] for i in range(N_CORES)], axis=0)
    return out, res


def kernel(**inputs) -> np.ndarray:
    out, _ = run(inputs)
    return out
